# revision 17
# baseline (speedup 1.0000x reference)
"""Trainium2 Bass kernel for nn_AttentionLayer (B=8, N=1024, D=256, H=4).

Sharding: pure data-parallel over batch B across 8 NeuronCores (one batch
element per core, all parameters replicated). No collectives.

Per-core algorithm (bf16 matmuls, fp32 stats/output):
  x_norm = LN(x)                               (bn_stats, free-dim LN)
  xnT    = x_norm^T                            (DMA xbar transpose, bf16)
  per head h:
    A_h  = wq_h @ wk_h^T                       (256x256 -> s = xn A xn^T)
    B^T  = [d', n] = A-contract vs xnT
    s^T  = xnT-chunk.T @ B^T                   ([m, n] logits in PSUM)
    esT  = exp(s^T/16)                         (ACT; already av-lhsT layout)
    v''  = [(xn @ wv_h) * m_k | m_k]           ([m, 257], key mask folded)
    out  = esT.T @ v''                         ([n, 257]; col 256 = denom)
    gate = sigmoid via tanh                    (ACT tanh + fused affine)
    t_h  = out * gate * (0.5*m_q/denom) + x    (fused scalar_tensor_tensor)
  z     = concat_h LN_lnr(t_h)   (gamma folded into out_w; beta via bias row)
  y     = zT.T @ out_w' + (out_b + beta_r@out_w) + x
  out   = LN_lno(y) * mask
Weights stream in via SWDGE cast-DMAs (f32->bf16 during transfer); wq/wk are
transposed on the tensor engine at startup (also warms HAM).
"""

import os
import sys

for _p in ("/opt/trn_rl_repo", "/root/.axon_site/_ro/trn_rl_repo"):
    if os.path.isdir(_p) and _p not in sys.path:
        sys.path.insert(0, _p)
        break

import numpy as np

N, D, H = 1024, 256, 4
NCH = N // 128  # 8 token chunks
EPS = 1e-6
SCALE = 1.0 / 16.0

_PROGRAM = None  # built Bass program, cached across kernel() calls


def _build_program():
    from contextlib import ExitStack

    import concourse.bass as bass
    import concourse.mybir as mybir
    import concourse.tile as tile
    from concourse import bacc
    from concourse.masks import make_identity

    f32 = mybir.dt.float32
    bf16 = mybir.dt.bfloat16
    i32 = mybir.dt.int32
    AF = mybir.ActivationFunctionType
    OP = mybir.AluOpType

    nc = bacc.Bacc(
        "TRN2",
        target_bir_lowering=False,
        debug=False,
        enable_asserts=False,
        num_devices=8,
    )

    x_d = nc.dram_tensor("x", [N, D], f32, kind="ExternalInput")
    mask_d = nc.dram_tensor("mask", [N], i32, kind="ExternalInput")
    wq_d = nc.dram_tensor("wq", [H, D, D], f32, kind="ExternalInput")
    wk_d = nc.dram_tensor("wk", [H, D, D], f32, kind="ExternalInput")
    wv_d = nc.dram_tensor("wv", [H, D, D], f32, kind="ExternalInput")
    wg_d = nc.dram_tensor("wg", [H, D, D], f32, kind="ExternalInput")
    ow_d = nc.dram_tensor("out_w", [D * H, D], f32, kind="ExternalInput")
    ob_d = nc.dram_tensor("out_b", [D], f32, kind="ExternalInput")
    lng_d = nc.dram_tensor("ln_g", [D], f32, kind="ExternalInput")
    lnb_d = nc.dram_tensor("ln_b", [D], f32, kind="ExternalInput")
    lnrg_d = nc.dram_tensor("lnr_g", [D], f32, kind="ExternalInput")
    lnrb_d = nc.dram_tensor("lnr_b", [D], f32, kind="ExternalInput")
    lnog_d = nc.dram_tensor("lno_g", [D], f32, kind="ExternalInput")
    lnob_d = nc.dram_tensor("lno_b", [D], f32, kind="ExternalInput")
    y_d = nc.dram_tensor("y", [N, D], f32, kind="ExternalOutput")

    def bcast_ap(ap, parts=128):
        return bass.AP(
            tensor=ap.tensor, offset=ap.offset, ap=[[0, parts]] + list(ap.ap)
        )

    with tile.TileContext(nc) as tc, ExitStack() as ctx:
        const = ctx.enter_context(tc.tile_pool(name="const", bufs=1))
        big = ctx.enter_context(tc.tile_pool(name="big", bufs=1))
        hpool = ctx.enter_context(tc.tile_pool(name="hpool", bufs=2))
        spool = ctx.enter_context(tc.tile_pool(name="spool", bufs=12))
        small = ctx.enter_context(tc.tile_pool(name="small", bufs=3))
        ps_s = ctx.enter_context(tc.tile_pool(name="ps_s", bufs=2, space="PSUM"))
        ps_o = ctx.enter_context(tc.tile_pool(name="ps_o", bufs=2, space="PSUM"))
        ps_vg = ctx.enter_context(tc.tile_pool(name="ps_vg", bufs=2, space="PSUM"))

        # ---- stage 0a: x / mask on the sync ring (per chunk: LN starts early)
        mask_i = const.tile([128, NCH], i32)
        nc.sync.dma_start(out=mask_i, in_=mask_d.ap().rearrange("(c p) -> p c", p=128))
        x_sb = const.tile([128, NCH, D], f32)
        for c in range(NCH):
            nc.sync.dma_start(
                out=x_sb[:, c, :], in_=x_d.ap()[128 * c : 128 * (c + 1), :]
            )

        eps_t = const.tile([128, 1], f32)
        nc.vector.memset(eps_t, EPS)
        zero_t = const.tile([128, 1], f32)
        nc.vector.memset(zero_t, 0.0)
        ident = const.tile([128, 128], bf16)
        make_identity(nc, ident)

        # weights stream in on SWDGE with f32->bf16 cast during the transfer
        wq_bf = const.tile([128, H, 2, D], bf16)
        wk_bf = const.tile([128, H, 2, D], bf16)
        wv_bf = const.tile([128, H, 2, D], bf16)
        wg_bf = const.tile([128, H, 2, D], bf16)
        for wd, wb_dst in ((wq_d, wq_bf), (wk_d, wk_bf), (wv_d, wv_bf), (wg_d, wg_bf)):
            nc.gpsimd.dma_start(
                out=wb_dst,
                in_=wd.ap().rearrange("h (c p) e -> p h c e", p=128),
            )

        mask_f = const.tile([128, NCH], f32)
        nc.vector.tensor_copy(out=mask_f, in_=mask_i)
        m_half = const.tile([128, NCH], f32)
        nc.vector.tensor_scalar_mul(m_half, mask_f, 0.5)
        mask_bf = const.tile([128, NCH], bf16)
        nc.any.tensor_copy(out=mask_bf, in_=mask_f)

        lng_bc = const.tile([128, D], f32)
        nc.scalar.dma_start(out=lng_bc, in_=bcast_ap(lng_d.ap()))
        lnb_bc = const.tile([128, D], f32)
        nc.scalar.dma_start(out=lnb_bc, in_=bcast_ap(lnb_d.ap()))

        # ---- stage 1: first layernorm + xnT (fully per-chunk pipelined,
        # transposes on the tensor engine: no DMA-xbar mode switches)
        xn_full = big.tile([128, NCH, D * H], bf16, tag="xz")
        xn = xn_full[:, :, 0:D]
        xnT = const.tile([128, 2, N], bf16)  # [p, dc, n] = xn^T[128*dc+p, n]
        x_bf = const.tile([128, NCH, D], bf16)
        for c in range(NCH):
            st6 = small.tile([128, 6], f32, tag="st6")
            nc.vector.bn_stats(out=st6, in_=x_sb[:, c, :])
            mv = small.tile([128, 2], f32, tag="mv")
            nc.vector.bn_aggr(out=mv, in_=st6)
            rs = small.tile([128, 1], f32, tag="rs")
            nc.scalar.activation(
                out=rs, in_=mv[:, 1:2], func=AF.Sqrt, bias=eps_t[:], scale=1.0
            )
            nc.vector.reciprocal(rs, rs)
            t1 = small.tile([128, D], bf16, tag="lnt")
            nc.vector.scalar_tensor_tensor(
                out=t1, in0=x_sb[:, c, :], scalar=mv[:, 0:1], in1=lng_bc,
                op0=OP.subtract, op1=OP.mult,
            )
            nc.vector.scalar_tensor_tensor(
                out=xn[:, c, :], in0=t1, scalar=rs, in1=lnb_bc,
                op0=OP.mult, op1=OP.add,
            )
            for dc in range(2):
                tr_ps = ps_vg.tile([128, 512], bf16, tag="pvg")
                nc.tensor.transpose(
                    tr_ps[:, 0:128], xn[:, c, 128 * dc : 128 * dc + 128], ident
                )
                nc.any.tensor_copy(
                    out=xnT[:, dc, 128 * c : 128 * c + 128], in_=tr_ps[:, 0:128]
                )
            nc.any.tensor_copy(out=x_bf[:, c, :], in_=x_sb[:, c, :])


        # ---- stage 2: heads
        t_all = big.tile([128, H, NCH, D], bf16, tag="tz")
        mv_r = big.tile([128, H, NCH, 2], f32)
        z = big.tile([128, NCH, D * H], bf16, tag="xz")  # [p(n), c, h*256+e]
        zT = big.tile([128, NCH, N], bf16)  # [p, kc, n] = z^T[128*kc+p, n]
        y_sb = big.tile([128, NCH, D], f32)
        y_out = big.tile([128, NCH, D], f32)

        def tail_chunk(c):
            # everything from lnr-normalize to the final masked LN for one
            # token chunk; called inside head 3's av loop so the final
            # projection matmuls interleave with the remaining av matmuls
            rs4 = small.tile([128, 4], f32, tag="rs4")
            nc.scalar.activation(
                out=rs4, in_=mv_r[:, :, c, 1], func=AF.Sqrt, bias=eps_t[:], scale=1.0
            )
            nc.vector.reciprocal(rs4, rs4)
            for h in range(H):
                nc.vector.tensor_scalar(
                    z[:, c, D * h : D * (h + 1)],
                    t_all[:, h, c, :],
                    mv_r[:, h, c, 0:1],
                    rs4[:, h : h + 1],
                    OP.subtract,
                    OP.mult,
                )
            eng = nc.sync if c % 2 == 0 else nc.scalar
            eng.dma_start_transpose(
                out=zT[:, :, 128 * c : 128 * c + 128], in_=z[:, c, :]
            )
            y_ps = ps_s.tile([128, D], f32, tag="s")
            for kc in range(NCH):
                nc.tensor.matmul(
                    y_ps,
                    lhsT=zT[:, kc, 128 * c : 128 * c + 128],
                    rhs=wo_bf[:, kc // 2, kc % 2, :],
                    start=(kc == 0),
                    stop=(kc == NCH - 1),
                )
            nc.any.tensor_add(y_sb[:, c, :], y_ps, xb[:, c, :])
            st6 = small.tile([128, 6], f32, tag="st6")
            nc.vector.bn_stats(out=st6, in_=y_sb[:, c, :])
            mvo = small.tile([128, 2], f32, tag="mv")
            nc.vector.bn_aggr(out=mvo, in_=st6)
            rso = small.tile([128, 1], f32, tag="rs")
            nc.scalar.activation(
                out=rso, in_=mvo[:, 1:2], func=AF.Sqrt, bias=eps_t[:], scale=1.0
            )
            nc.vector.reciprocal(rso, rso)
            f1 = small.tile([128, D], f32, tag="f1")
            nc.vector.scalar_tensor_tensor(
                out=f1, in0=y_sb[:, c, :], scalar=mvo[:, 0:1], in1=lnog_bc,
                op0=OP.subtract, op1=OP.mult,
            )
            f2 = small.tile([128, D], f32, tag="f2")
            nc.vector.scalar_tensor_tensor(
                out=f2, in0=f1, scalar=rso, in1=lnob_bc,
                op0=OP.mult, op1=OP.add,
            )
            nc.any.tensor_scalar(
                y_out[:, c, :], f2, mask_f[:, c : c + 1], None, OP.mult
            )

        for h in range(H):
            # q^T, k^T = [e, n] projections (weights stay natural: no
            # weight transposes needed)
            qT_bf = hpool.tile([128, 2, N], bf16, tag="qT")
            kT_bf = hpool.tile([128, 2, N], bf16, tag="kT")
            for wsrc, tdst in ((wq_bf, qT_bf), (wk_bf, kT_bf)):
                for ec in range(2):
                    for nh in range(2):
                        p_ps = ps_vg.tile([128, 512], f32, tag="pvg")
                        for kd in range(2):
                            nc.tensor.matmul(
                                p_ps,
                                lhsT=wsrc[:, h, kd, 128 * ec : 128 * ec + 128],
                                rhs=xnT[:, kd, 512 * nh : 512 * nh + 512],
                                start=(kd == 0),
                                stop=(kd == 1),
                            )
                        nc.any.tensor_copy(
                            out=tdst[:, ec, 512 * nh : 512 * nh + 512], in_=p_ps
                        )

            # v'' = [xn @ wv * m_k | m_k]
            v2 = hpool.tile([128, NCH, D + 1], bf16, tag="v2")
            for mc in range(NCH):
                v_ps = ps_vg.tile([128, 512], f32, tag="pvg")
                for kd in range(2):
                    nc.tensor.matmul(
                        v_ps[:, 0:D],
                        lhsT=xnT[:, kd, 128 * mc : 128 * mc + 128],
                        rhs=wv_bf[:, h, kd, :],
                        start=(kd == 0),
                        stop=(kd == 1),
                    )
                nc.any.tensor_scalar(
                    v2[:, mc, 0:D], v_ps[:, 0:D], mask_f[:, mc : mc + 1], None, OP.mult
                )
            nc.any.tensor_copy(out=v2[:, :, D], in_=mask_bf)

            # gate pre-activation: tanh(0.5 * xn @ wg)
            tanh_o = hpool.tile([128, NCH, D], bf16, tag="tanh")
            for c in range(NCH):
                g_ps = ps_vg.tile([128, 512], f32, tag="pvg")
                for kd in range(2):
                    nc.tensor.matmul(
                        g_ps[:, 0:D],
                        lhsT=xnT[:, kd, 128 * c : 128 * c + 128],
                        rhs=wg_bf[:, h, kd, :],
                        start=(kd == 0),
                        stop=(kd == 1),
                    )
                nc.scalar.activation(
                    out=tanh_o[:, c, :], in_=g_ps[:, 0:D], func=AF.Tanh,
                    bias=zero_t[:], scale=0.5,
                )

            # logits transposed: s^T tiles [m-chunk, n]; exp output is the
            # av lhsT layout directly (no transpose)
            esT_tiles = []
            for mc in range(NCH):
                s_ps = ps_s.tile([128, N], f32, tag="s")
                for kc in range(2):
                    for nh in range(2):
                        nc.tensor.matmul(
                            s_ps[:, 512 * nh : 512 * nh + 512],
                            lhsT=kT_bf[:, kc, 128 * mc : 128 * mc + 128],
                            rhs=qT_bf[:, kc, 512 * nh : 512 * nh + 512],
                            start=(kc == 0),
                            stop=(kc == 1),
                        )
                esT = spool.tile([128, N], bf16, tag="esT")
                nc.scalar.activation(
                    out=esT, in_=s_ps, func=AF.Exp, bias=zero_t[:], scale=SCALE
                )
                esT_tiles.append(esT)

            for c in range(NCH):
                o_ps = ps_o.tile([128, D + 1], f32, tag="o")
                for mc in range(NCH):
                    nc.tensor.matmul(
                        o_ps,
                        lhsT=esT_tiles[mc][:, 128 * c : 128 * c + 128],
                        rhs=v2[:, mc, :],
                        start=(mc == 0),
                        stop=(mc == NCH - 1),
                    )
                hf = small.tile([128, 1], f32, tag="hf")
                nc.vector.reciprocal(hf, o_ps[:, D : D + 1])
                nc.vector.tensor_scalar_mul(hf, hf, m_half[:, c : c + 1])
                tmp = small.tile([128, D], bf16, tag="tmp")
                nc.vector.scalar_tensor_tensor(
                    out=tmp,
                    in0=tanh_o[:, c, :],
                    scalar=1.0,
                    in1=o_ps[:, 0:D],
                    op0=OP.add,
                    op1=OP.mult,
                )
                nc.vector.scalar_tensor_tensor(
                    out=t_all[:, h, c, :],
                    in0=tmp,
                    scalar=hf,
                    in1=x_bf[:, c, :],
                    op0=OP.mult,
                    op1=OP.add,
                )
                st6 = small.tile([128, 6], f32, tag="st6")
                nc.vector.bn_stats(out=st6, in_=t_all[:, h, c, :])
                nc.vector.bn_aggr(out=mv_r[:, h, c, :], in_=st6)
                if h == H - 1:
                    tail_chunk(c)

            if h == 1:
                # out_w / bias prep emitted mid-kernel: DMAs overlap head
                # compute, results only needed at the tail
                gcol = const.tile([128, 2], f32)
                nc.gpsimd.dma_start(
                    out=gcol, in_=lnrg_d.ap().rearrange("(b p) -> p b", p=128)
                )
                bcol_bf = const.tile([128, 2], bf16)
                nc.gpsimd.dma_start(
                    out=bcol_bf, in_=lnrb_d.ap().rearrange("(b p) -> p b", p=128)
                )
                # out_w permuted to [p, h, b, col] (row (128b+p)*4+h), bf16 cast
                wo_raw = const.tile([128, H, 2, D], bf16)
                nc.gpsimd.dma_start(
                    out=wo_raw,
                    in_=ow_d.ap().rearrange("(b p h) o -> p h b o", b=2, p=128, h=H),
                )
                wo_bf = const.tile([128, H, 2, D], bf16)
                for hh in range(H):
                    for b2 in range(2):
                        nc.any.tensor_scalar(
                            wo_bf[:, hh, b2, :],
                            wo_raw[:, hh, b2, :],
                            gcol[:, b2 : b2 + 1],
                            None,
                            OP.mult,
                        )
                ob_row = const.tile([1, D], f32)
                ob_ap = ob_d.ap()
                nc.gpsimd.dma_start(
                    out=ob_row,
                    in_=bass.AP(
                        tensor=ob_ap.tensor, offset=ob_ap.offset,
                        ap=[[0, 1]] + list(ob_ap.ap),
                    ),
                )
                lnog_bc = const.tile([128, D], f32)
                nc.gpsimd.dma_start(out=lnog_bc, in_=bcast_ap(lnog_d.ap()))
                lnob_bc = const.tile([128, D], f32)
                nc.gpsimd.dma_start(out=lnob_bc, in_=bcast_ap(lnob_d.ap()))

            if h == 2:
                # bias row = out_b + lnr_b @ out_w, broadcast via DRAM
                bias_ps = ps_o.tile([1, D], f32, tag="o")
                i = 0
                for b2 in range(2):
                    for hh in range(H):
                        nc.tensor.matmul(
                            bias_ps,
                            lhsT=bcol_bf[:, b2 : b2 + 1],
                            rhs=wo_raw[:, hh, b2, :],
                            start=(i == 0),
                            stop=(i == 7),
                        )
                        i += 1
                bias_row = const.tile([1, D], f32)
                nc.vector.tensor_add(bias_row, bias_ps, ob_row)
                bias_dram = nc.dram_tensor("bias_scratch", [D], f32, kind="Internal")
                nc.gpsimd.dma_start(
                    out=bias_dram.ap().rearrange("(o d) -> o d", o=1), in_=bias_row
                )
                bias_bc = const.tile([128, D], f32)
                nc.gpsimd.dma_start(out=bias_bc, in_=bcast_ap(bias_dram.ap()))
                xb = const.tile([128, NCH, D], f32)
                for c in range(NCH):
                    nc.any.tensor_add(xb[:, c, :], x_sb[:, c, :], bias_bc)

        nc.sync.dma_start(
            out=y_d.ap().rearrange("(c p) d -> p c d", p=128), in_=y_out
        )

    nc.compile()
    return nc


def _get_program():
    global _PROGRAM
    if _PROGRAM is None:
        _PROGRAM = _build_program()
    return _PROGRAM


def _make_in_maps(inputs):
    full = {k: np.asarray(v) for k, v in inputs.items()}
    in_maps = []
    for b in range(8):
        m = {
            "x": np.ascontiguousarray(full["x"][b], dtype=np.float32),
            "mask": np.ascontiguousarray(full["mask"][b], dtype=np.int32),
        }
        for k in ("wq", "wk", "wv", "wg", "out_w", "out_b", "ln_g", "ln_b",
                  "lnr_g", "lnr_b", "lno_g", "lno_b"):
            m[k] = np.ascontiguousarray(full[k], dtype=np.float32)
        in_maps.append(m)
    return in_maps


def run_on_hw(inputs, trace=False):
    """Run on the 8 NeuronCores; returns (output [8,1024,256] f32, results obj)."""
    from concourse import bass_utils

    nc = _get_program()
    in_maps = _make_in_maps(inputs)
    res = bass_utils.run_bass_kernel_spmd(
        nc, in_maps, core_ids=list(range(8)), trace=trace
    )
    out = np.stack([res.results[b]["y"] for b in range(8)], axis=0).astype(np.float32)
    return out, res


def kernel(**inputs) -> np.ndarray:
    out, _ = run_on_hw(inputs, trace=False)
    return out


# revision 18
# speedup vs baseline: 1.2441x; 1.2441x over previous
"""Trainium2 Bass kernel for nn_AttentionLayer (B=8, N=1024, D=256, H=4).

Sharding: pure data-parallel over batch B across 8 NeuronCores (one batch
element per core, all parameters replicated). No collectives.

Per-core algorithm (bf16 matmuls, fp32 stats/output):
  x_norm = LN(x)                               (bn_stats, free-dim LN)
  xnT    = x_norm^T                            (DMA xbar transpose, bf16)
  per head h:
    A_h  = wq_h @ wk_h^T                       (256x256 -> s = xn A xn^T)
    B^T  = [d', n] = A-contract vs xnT
    s^T  = xnT-chunk.T @ B^T                   ([m, n] logits in PSUM)
    esT  = exp(s^T/16)                         (ACT; already av-lhsT layout)
    v''  = [(xn @ wv_h) * m_k | m_k]           ([m, 257], key mask folded)
    out  = esT.T @ v''                         ([n, 257]; col 256 = denom)
    gate = sigmoid via tanh                    (ACT tanh + fused affine)
    t_h  = out * gate * (0.5*m_q/denom) + x    (fused scalar_tensor_tensor)
  z     = concat_h LN_lnr(t_h)   (gamma folded into out_w; beta via bias row)
  y     = zT.T @ out_w' + (out_b + beta_r@out_w) + x
  out   = LN_lno(y) * mask
Weights stream in via SWDGE cast-DMAs (f32->bf16 during transfer); wq/wk are
transposed on the tensor engine at startup (also warms HAM).
"""

import os
import sys

for _p in ("/opt/trn_rl_repo", "/root/.axon_site/_ro/trn_rl_repo"):
    if os.path.isdir(_p) and _p not in sys.path:
        sys.path.insert(0, _p)
        break

import numpy as np

N, D, H = 1024, 256, 4
NCH = N // 128  # 8 token chunks
EPS = 1e-6
SCALE = 1.0 / 16.0

_PROGRAM = None  # built Bass program, cached across kernel() calls


def _build_program():
    from contextlib import ExitStack

    import concourse.bass as bass
    import concourse.mybir as mybir
    import concourse.tile as tile
    from concourse import bacc
    from concourse.masks import make_identity

    f32 = mybir.dt.float32
    bf16 = mybir.dt.bfloat16
    i32 = mybir.dt.int32
    AF = mybir.ActivationFunctionType
    OP = mybir.AluOpType

    nc = bacc.Bacc(
        "TRN2",
        target_bir_lowering=False,
        debug=False,
        enable_asserts=False,
        num_devices=8,
    )

    x_d = nc.dram_tensor("x", [N, D], f32, kind="ExternalInput")
    mask_d = nc.dram_tensor("mask", [N], i32, kind="ExternalInput")
    wq_d = nc.dram_tensor("wq", [H, D, D], f32, kind="ExternalInput")
    wk_d = nc.dram_tensor("wk", [H, D, D], f32, kind="ExternalInput")
    wv_d = nc.dram_tensor("wv", [H, D, D], f32, kind="ExternalInput")
    wg_d = nc.dram_tensor("wg", [H, D, D], f32, kind="ExternalInput")
    ow_d = nc.dram_tensor("out_w", [D * H, D], f32, kind="ExternalInput")
    ob_d = nc.dram_tensor("out_b", [D], f32, kind="ExternalInput")
    lng_d = nc.dram_tensor("ln_g", [D], f32, kind="ExternalInput")
    lnb_d = nc.dram_tensor("ln_b", [D], f32, kind="ExternalInput")
    lnrg_d = nc.dram_tensor("lnr_g", [D], f32, kind="ExternalInput")
    lnrb_d = nc.dram_tensor("lnr_b", [D], f32, kind="ExternalInput")
    lnog_d = nc.dram_tensor("lno_g", [D], f32, kind="ExternalInput")
    lnob_d = nc.dram_tensor("lno_b", [D], f32, kind="ExternalInput")
    y_d = nc.dram_tensor("y", [N, D], f32, kind="ExternalOutput")

    def bcast_ap(ap, parts=128):
        return bass.AP(
            tensor=ap.tensor, offset=ap.offset, ap=[[0, parts]] + list(ap.ap)
        )

    with tile.TileContext(nc) as tc, ExitStack() as ctx:
        const = ctx.enter_context(tc.tile_pool(name="const", bufs=1))
        big = ctx.enter_context(tc.tile_pool(name="big", bufs=1))
        hpool = ctx.enter_context(tc.tile_pool(name="hpool", bufs=2))
        spool = ctx.enter_context(tc.tile_pool(name="spool", bufs=12))
        small = ctx.enter_context(tc.tile_pool(name="small", bufs=3))
        ps_s = ctx.enter_context(tc.tile_pool(name="ps_s", bufs=2, space="PSUM"))
        ps_o = ctx.enter_context(tc.tile_pool(name="ps_o", bufs=2, space="PSUM"))
        ps_vg = ctx.enter_context(tc.tile_pool(name="ps_vg", bufs=2, space="PSUM"))

        # ---- stage 0a: x / mask on the sync ring (per chunk: LN starts early)
        mask_i = const.tile([128, NCH], i32)
        nc.sync.dma_start(out=mask_i, in_=mask_d.ap().rearrange("(c p) -> p c", p=128))
        x_sb = const.tile([128, NCH, D], f32)
        for c in range(NCH):
            nc.sync.dma_start(
                out=x_sb[:, c, :], in_=x_d.ap()[128 * c : 128 * (c + 1), :]
            )

        eps_t = const.tile([128, 1], f32)
        nc.vector.memset(eps_t, EPS)
        zero_t = const.tile([128, 1], f32)
        nc.vector.memset(zero_t, 0.0)
        ident = const.tile([128, 128], bf16)
        make_identity(nc, ident)

        # weights stream in on SWDGE with f32->bf16 cast during the transfer
        wq_bf = const.tile([128, H, 2, D], bf16)
        wk_bf = const.tile([128, H, 2, D], bf16)
        wv_bf = const.tile([128, H, 2, D], bf16)
        wg_bf = const.tile([128, H, 2, D], bf16)
        for wd, wb_dst in ((wq_d, wq_bf), (wk_d, wk_bf), (wv_d, wv_bf), (wg_d, wg_bf)):
            nc.gpsimd.dma_start(
                out=wb_dst,
                in_=wd.ap().rearrange("h (c p) e -> p h c e", p=128),
            )

        mask_f = const.tile([128, NCH], f32)
        nc.vector.tensor_copy(out=mask_f, in_=mask_i)
        m_half = const.tile([128, NCH], f32)
        nc.vector.tensor_scalar_mul(m_half, mask_f, 0.5)
        mask_bf = const.tile([128, NCH], bf16)
        nc.any.tensor_copy(out=mask_bf, in_=mask_f)

        lng_bc = const.tile([128, D], f32)
        nc.scalar.dma_start(out=lng_bc, in_=bcast_ap(lng_d.ap()))
        lnb_bc = const.tile([128, D], f32)
        nc.scalar.dma_start(out=lnb_bc, in_=bcast_ap(lnb_d.ap()))

        # ---- stage 1: first layernorm + xnT (fully per-chunk pipelined,
        # transposes on the tensor engine: no DMA-xbar mode switches)
        xn_full = big.tile([128, NCH, D * H], bf16, tag="xz")
        xn = xn_full[:, :, 0:D]
        xnT = const.tile([128, 2, N], bf16)  # [p, dc, n] = xn^T[128*dc+p, n]
        x_bf = const.tile([128, NCH, D], bf16)
        for c in range(NCH):
            st6 = small.tile([128, 6], f32, tag="st6")
            nc.vector.bn_stats(out=st6, in_=x_sb[:, c, :])
            mv = small.tile([128, 2], f32, tag="mv")
            nc.vector.bn_aggr(out=mv, in_=st6)
            rs = small.tile([128, 1], f32, tag="rs")
            nc.scalar.activation(
                out=rs, in_=mv[:, 1:2], func=AF.Sqrt, bias=eps_t[:], scale=1.0
            )
            nc.vector.reciprocal(rs, rs)
            t1 = small.tile([128, D], bf16, tag="lnt")
            nc.vector.scalar_tensor_tensor(
                out=t1, in0=x_sb[:, c, :], scalar=mv[:, 0:1], in1=lng_bc,
                op0=OP.subtract, op1=OP.mult,
            )
            nc.vector.scalar_tensor_tensor(
                out=xn[:, c, :], in0=t1, scalar=rs, in1=lnb_bc,
                op0=OP.mult, op1=OP.add,
            )
            for dc in range(2):
                tr_ps = ps_vg.tile([128, 512], bf16, tag="pvg")
                nc.tensor.transpose(
                    tr_ps[:, 0:128], xn[:, c, 128 * dc : 128 * dc + 128], ident
                )
                nc.any.tensor_copy(
                    out=xnT[:, dc, 128 * c : 128 * c + 128], in_=tr_ps[:, 0:128]
                )
            nc.any.tensor_copy(out=x_bf[:, c, :], in_=x_sb[:, c, :])


        # ---- stage 2: heads
        t_all = big.tile([128, H, NCH, D], bf16, tag="tz")
        mv_r = big.tile([128, H, NCH, 2], f32)
        z = big.tile([128, NCH, D * H], bf16, tag="xz")  # [p(n), c, h*256+e]
        zT = big.tile([128, NCH, N], bf16)  # [p, kc, n] = z^T[128*kc+p, n]
        y_sb = big.tile([128, NCH, D], f32)
        y_out = big.tile([128, NCH, D], f32)

        def tail_prep_chunk(c):
            # lnr-normalize + transpose for one token chunk; called inside
            # head 3's av loop so this DVE/DMA work hides under av matmuls
            rs4 = small.tile([128, 4], f32, tag="rs4")
            nc.scalar.activation(
                out=rs4, in_=mv_r[:, :, c, 1], func=AF.Sqrt, bias=eps_t[:], scale=1.0
            )
            nc.vector.reciprocal(rs4, rs4)
            for h in range(H):
                nc.vector.tensor_scalar(
                    z[:, c, D * h : D * (h + 1)],
                    t_all[:, h, c, :],
                    mv_r[:, h, c, 0:1],
                    rs4[:, h : h + 1],
                    OP.subtract,
                    OP.mult,
                )
            eng = nc.sync if c % 2 == 0 else nc.scalar
            eng.dma_start_transpose(
                out=zT[:, :, 128 * c : 128 * c + 128], in_=z[:, c, :]
            )

        for h in range(H):
            # q^T, k^T = [e, n] projections (weights stay natural: no
            # weight transposes needed)
            qT_bf = hpool.tile([128, 2, N], bf16, tag="qT")
            kT_bf = hpool.tile([128, 2, N], bf16, tag="kT")
            for wsrc, tdst in ((wq_bf, qT_bf), (wk_bf, kT_bf)):
                for ec in range(2):
                    for nh in range(2):
                        p_ps = ps_vg.tile([128, 512], f32, tag="pvg")
                        for kd in range(2):
                            nc.tensor.matmul(
                                p_ps,
                                lhsT=wsrc[:, h, kd, 128 * ec : 128 * ec + 128],
                                rhs=xnT[:, kd, 512 * nh : 512 * nh + 512],
                                start=(kd == 0),
                                stop=(kd == 1),
                            )
                        nc.any.tensor_copy(
                            out=tdst[:, ec, 512 * nh : 512 * nh + 512], in_=p_ps
                        )

            # v'' = [xn @ wv * m_k | m_k]
            v2 = hpool.tile([128, NCH, D + 1], bf16, tag="v2")
            for mc in range(NCH):
                v_ps = ps_vg.tile([128, 512], f32, tag="pvg")
                for kd in range(2):
                    nc.tensor.matmul(
                        v_ps[:, 0:D],
                        lhsT=xnT[:, kd, 128 * mc : 128 * mc + 128],
                        rhs=wv_bf[:, h, kd, :],
                        start=(kd == 0),
                        stop=(kd == 1),
                    )
                nc.any.tensor_scalar(
                    v2[:, mc, 0:D], v_ps[:, 0:D], mask_f[:, mc : mc + 1], None, OP.mult
                )
            nc.any.tensor_copy(out=v2[:, :, D], in_=mask_bf)

            # gate pre-activation: tanh(0.5 * xn @ wg)
            tanh_o = hpool.tile([128, NCH, D], bf16, tag="tanh")
            for c in range(NCH):
                g_ps = ps_vg.tile([128, 512], f32, tag="pvg")
                for kd in range(2):
                    nc.tensor.matmul(
                        g_ps[:, 0:D],
                        lhsT=xnT[:, kd, 128 * c : 128 * c + 128],
                        rhs=wg_bf[:, h, kd, :],
                        start=(kd == 0),
                        stop=(kd == 1),
                    )
                nc.scalar.activation(
                    out=tanh_o[:, c, :], in_=g_ps[:, 0:D], func=AF.Tanh,
                    bias=zero_t[:], scale=0.5,
                )

            # logits transposed: s^T tiles [m-chunk, n]; exp output is the
            # av lhsT layout directly (no transpose)
            esT_tiles = []
            for mc in range(NCH):
                s_ps = ps_s.tile([128, N], f32, tag="s")
                for kc in range(2):
                    for nh in range(2):
                        nc.tensor.matmul(
                            s_ps[:, 512 * nh : 512 * nh + 512],
                            lhsT=kT_bf[:, kc, 128 * mc : 128 * mc + 128],
                            rhs=qT_bf[:, kc, 512 * nh : 512 * nh + 512],
                            start=(kc == 0),
                            stop=(kc == 1),
                        )
                esT = spool.tile([128, N], bf16, tag="esT")
                nc.scalar.activation(
                    out=esT, in_=s_ps, func=AF.Exp, bias=zero_t[:], scale=SCALE
                )
                esT_tiles.append(esT)

            for c in range(NCH):
                o_ps = ps_o.tile([128, D + 1], f32, tag="o")
                for mc in range(NCH):
                    nc.tensor.matmul(
                        o_ps,
                        lhsT=esT_tiles[mc][:, 128 * c : 128 * c + 128],
                        rhs=v2[:, mc, :],
                        start=(mc == 0),
                        stop=(mc == NCH - 1),
                    )
                hf = small.tile([128, 1], f32, tag="hf")
                nc.vector.reciprocal(hf, o_ps[:, D : D + 1])
                nc.vector.tensor_scalar_mul(hf, hf, m_half[:, c : c + 1])
                tmp = small.tile([128, D], bf16, tag="tmp")
                nc.vector.scalar_tensor_tensor(
                    out=tmp,
                    in0=tanh_o[:, c, :],
                    scalar=1.0,
                    in1=o_ps[:, 0:D],
                    op0=OP.add,
                    op1=OP.mult,
                )
                nc.vector.scalar_tensor_tensor(
                    out=t_all[:, h, c, :],
                    in0=tmp,
                    scalar=hf,
                    in1=x_bf[:, c, :],
                    op0=OP.mult,
                    op1=OP.add,
                )
                st6 = small.tile([128, 6], f32, tag="st6")
                nc.vector.bn_stats(out=st6, in_=t_all[:, h, c, :])
                nc.vector.bn_aggr(out=mv_r[:, h, c, :], in_=st6)
                if h == H - 1:
                    tail_prep_chunk(c)

            if h == 1:
                # out_w / bias prep emitted mid-kernel: DMAs overlap head
                # compute, results only needed at the tail
                gcol = const.tile([128, 2], f32)
                nc.gpsimd.dma_start(
                    out=gcol, in_=lnrg_d.ap().rearrange("(b p) -> p b", p=128)
                )
                bcol_bf = const.tile([128, 2], bf16)
                nc.gpsimd.dma_start(
                    out=bcol_bf, in_=lnrb_d.ap().rearrange("(b p) -> p b", p=128)
                )
                # out_w permuted to [p, h, b, col] (row (128b+p)*4+h), bf16 cast
                wo_raw = const.tile([128, H, 2, D], bf16)
                nc.gpsimd.dma_start(
                    out=wo_raw,
                    in_=ow_d.ap().rearrange("(b p h) o -> p h b o", b=2, p=128, h=H),
                )
                wo_bf = const.tile([128, H, 2, D], bf16)
                for hh in range(H):
                    for b2 in range(2):
                        nc.any.tensor_scalar(
                            wo_bf[:, hh, b2, :],
                            wo_raw[:, hh, b2, :],
                            gcol[:, b2 : b2 + 1],
                            None,
                            OP.mult,
                        )
                ob_row = const.tile([1, D], f32)
                ob_ap = ob_d.ap()
                nc.gpsimd.dma_start(
                    out=ob_row,
                    in_=bass.AP(
                        tensor=ob_ap.tensor, offset=ob_ap.offset,
                        ap=[[0, 1]] + list(ob_ap.ap),
                    ),
                )
                lnog_bc = const.tile([128, D], f32)
                nc.gpsimd.dma_start(out=lnog_bc, in_=bcast_ap(lnog_d.ap()))
                lnob_bc = const.tile([128, D], f32)
                nc.gpsimd.dma_start(out=lnob_bc, in_=bcast_ap(lnob_d.ap()))

            if h == 2:
                # bias row = out_b + lnr_b @ out_w, broadcast via DRAM
                bias_ps = ps_o.tile([1, D], f32, tag="o")
                i = 0
                for b2 in range(2):
                    for hh in range(H):
                        nc.tensor.matmul(
                            bias_ps,
                            lhsT=bcol_bf[:, b2 : b2 + 1],
                            rhs=wo_raw[:, hh, b2, :],
                            start=(i == 0),
                            stop=(i == 7),
                        )
                        i += 1
                bias_row = const.tile([1, D], f32)
                nc.vector.tensor_add(bias_row, bias_ps, ob_row)
                bias_dram = nc.dram_tensor("bias_scratch", [D], f32, kind="Internal")
                nc.gpsimd.dma_start(
                    out=bias_dram.ap().rearrange("(o d) -> o d", o=1), in_=bias_row
                )
                bias_bc = const.tile([128, D], f32)
                nc.gpsimd.dma_start(out=bias_bc, in_=bcast_ap(bias_dram.ap()))
                xb = const.tile([128, NCH, D], f32)
                for c in range(NCH):
                    nc.any.tensor_add(xb[:, c, :], x_sb[:, c, :], bias_bc)

        for c in range(NCH):
            y_ps = ps_s.tile([128, D], f32, tag="s")
            for kc in range(NCH):
                nc.tensor.matmul(
                    y_ps,
                    lhsT=zT[:, kc, 128 * c : 128 * c + 128],
                    rhs=wo_bf[:, kc // 2, kc % 2, :],
                    start=(kc == 0),
                    stop=(kc == NCH - 1),
                )
            nc.any.tensor_add(y_sb[:, c, :], y_ps, xb[:, c, :])
            st6 = small.tile([128, 6], f32, tag="st6")
            nc.vector.bn_stats(out=st6, in_=y_sb[:, c, :])
            mvo = small.tile([128, 2], f32, tag="mv")
            nc.vector.bn_aggr(out=mvo, in_=st6)
            rso = small.tile([128, 1], f32, tag="rs")
            nc.scalar.activation(
                out=rso, in_=mvo[:, 1:2], func=AF.Sqrt, bias=eps_t[:], scale=1.0
            )
            nc.vector.reciprocal(rso, rso)
            f1 = small.tile([128, D], f32, tag="f1")
            nc.vector.scalar_tensor_tensor(
                out=f1, in0=y_sb[:, c, :], scalar=mvo[:, 0:1], in1=lnog_bc,
                op0=OP.subtract, op1=OP.mult,
            )
            f2 = small.tile([128, D], f32, tag="f2")
            nc.vector.scalar_tensor_tensor(
                out=f2, in0=f1, scalar=rso, in1=lnob_bc,
                op0=OP.mult, op1=OP.add,
            )
            nc.any.tensor_scalar(
                y_out[:, c, :], f2, mask_f[:, c : c + 1], None, OP.mult
            )
        nc.sync.dma_start(
            out=y_d.ap().rearrange("(c p) d -> p c d", p=128), in_=y_out
        )

    nc.compile()
    return nc


def _get_program():
    global _PROGRAM
    if _PROGRAM is None:
        _PROGRAM = _build_program()
    return _PROGRAM


def _make_in_maps(inputs):
    full = {k: np.asarray(v) for k, v in inputs.items()}
    in_maps = []
    for b in range(8):
        m = {
            "x": np.ascontiguousarray(full["x"][b], dtype=np.float32),
            "mask": np.ascontiguousarray(full["mask"][b], dtype=np.int32),
        }
        for k in ("wq", "wk", "wv", "wg", "out_w", "out_b", "ln_g", "ln_b",
                  "lnr_g", "lnr_b", "lno_g", "lno_b"):
            m[k] = np.ascontiguousarray(full[k], dtype=np.float32)
        in_maps.append(m)
    return in_maps


def run_on_hw(inputs, trace=False):
    """Run on the 8 NeuronCores; returns (output [8,1024,256] f32, results obj)."""
    from concourse import bass_utils

    nc = _get_program()
    in_maps = _make_in_maps(inputs)
    res = bass_utils.run_bass_kernel_spmd(
        nc, in_maps, core_ids=list(range(8)), trace=trace
    )
    out = np.stack([res.results[b]["y"] for b in range(8)], axis=0).astype(np.float32)
    return out, res


def kernel(**inputs) -> np.ndarray:
    out, _ = run_on_hw(inputs, trace=False)
    return out


# revision 19
# speedup vs baseline: 1.2557x; 1.0093x over previous
"""Trainium2 Bass kernel for nn_AttentionLayer (B=8, N=1024, D=256, H=4).

Sharding: pure data-parallel over batch B across 8 NeuronCores (one batch
element per core, all parameters replicated). No collectives.

Per-core algorithm (bf16 matmuls, fp32 stats/output):
  x_norm = LN(x)                               (bn_stats, free-dim LN)
  xnT    = x_norm^T                            (DMA xbar transpose, bf16)
  per head h:
    A_h  = wq_h @ wk_h^T                       (256x256 -> s = xn A xn^T)
    B^T  = [d', n] = A-contract vs xnT
    s^T  = xnT-chunk.T @ B^T                   ([m, n] logits in PSUM)
    esT  = exp(s^T/16)                         (ACT; already av-lhsT layout)
    v''  = [(xn @ wv_h) * m_k | m_k]           ([m, 257], key mask folded)
    out  = esT.T @ v''                         ([n, 257]; col 256 = denom)
    gate = sigmoid via tanh                    (ACT tanh + fused affine)
    t_h  = out * gate * (0.5*m_q/denom) + x    (fused scalar_tensor_tensor)
  z     = concat_h LN_lnr(t_h)   (gamma folded into out_w; beta via bias row)
  y     = zT.T @ out_w' + (out_b + beta_r@out_w) + x
  out   = LN_lno(y) * mask
Weights stream in via SWDGE cast-DMAs (f32->bf16 during transfer); wq/wk are
transposed on the tensor engine at startup (also warms HAM).
"""

import os
import sys

for _p in ("/opt/trn_rl_repo", "/root/.axon_site/_ro/trn_rl_repo"):
    if os.path.isdir(_p) and _p not in sys.path:
        sys.path.insert(0, _p)
        break

import numpy as np

N, D, H = 1024, 256, 4
NCH = N // 128  # 8 token chunks
EPS = 1e-6
SCALE = 1.0 / 16.0

_PROGRAM = None  # built Bass program, cached across kernel() calls


def _build_program():
    from contextlib import ExitStack

    import concourse.bass as bass
    import concourse.mybir as mybir
    import concourse.tile as tile
    from concourse import bacc
    from concourse.masks import make_identity

    f32 = mybir.dt.float32
    bf16 = mybir.dt.bfloat16
    i32 = mybir.dt.int32
    AF = mybir.ActivationFunctionType
    OP = mybir.AluOpType

    nc = bacc.Bacc(
        "TRN2",
        target_bir_lowering=False,
        debug=False,
        enable_asserts=False,
        num_devices=8,
    )

    x_d = nc.dram_tensor("x", [N, D], f32, kind="ExternalInput")
    mask_d = nc.dram_tensor("mask", [N], i32, kind="ExternalInput")
    wq_d = nc.dram_tensor("wq", [H, D, D], f32, kind="ExternalInput")
    wk_d = nc.dram_tensor("wk", [H, D, D], f32, kind="ExternalInput")
    wv_d = nc.dram_tensor("wv", [H, D, D], f32, kind="ExternalInput")
    wg_d = nc.dram_tensor("wg", [H, D, D], f32, kind="ExternalInput")
    ow_d = nc.dram_tensor("out_w", [D * H, D], f32, kind="ExternalInput")
    ob_d = nc.dram_tensor("out_b", [D], f32, kind="ExternalInput")
    lng_d = nc.dram_tensor("ln_g", [D], f32, kind="ExternalInput")
    lnb_d = nc.dram_tensor("ln_b", [D], f32, kind="ExternalInput")
    lnrg_d = nc.dram_tensor("lnr_g", [D], f32, kind="ExternalInput")
    lnrb_d = nc.dram_tensor("lnr_b", [D], f32, kind="ExternalInput")
    lnog_d = nc.dram_tensor("lno_g", [D], f32, kind="ExternalInput")
    lnob_d = nc.dram_tensor("lno_b", [D], f32, kind="ExternalInput")
    y_d = nc.dram_tensor("y", [N, D], f32, kind="ExternalOutput")

    def bcast_ap(ap, parts=128):
        return bass.AP(
            tensor=ap.tensor, offset=ap.offset, ap=[[0, parts]] + list(ap.ap)
        )

    with tile.TileContext(nc) as tc, ExitStack() as ctx:
        const = ctx.enter_context(tc.tile_pool(name="const", bufs=1))
        big = ctx.enter_context(tc.tile_pool(name="big", bufs=1))
        hpool = ctx.enter_context(tc.tile_pool(name="hpool", bufs=2))
        spool = ctx.enter_context(tc.tile_pool(name="spool", bufs=12))
        small = ctx.enter_context(tc.tile_pool(name="small", bufs=3))
        ps_s = ctx.enter_context(tc.tile_pool(name="ps_s", bufs=2, space="PSUM"))
        ps_o = ctx.enter_context(tc.tile_pool(name="ps_o", bufs=2, space="PSUM"))
        ps_vg = ctx.enter_context(tc.tile_pool(name="ps_vg", bufs=2, space="PSUM"))

        # ---- stage 0a: x / mask on the sync ring (per chunk: LN starts early)
        # x first on the single SWDGE FIFO ring (it gates the LN -> xnT ->
        # projections critical path), weights immediately behind, mask last
        x_sb = const.tile([128, NCH, D], f32)
        for half in range(2):
            nc.gpsimd.dma_start(
                out=x_sb[:, 4 * half : 4 * half + 4, :],
                in_=x_d.ap()[512 * half : 512 * (half + 1), :].rearrange(
                    "(c p) d -> p c d", p=128
                ),
            )
        wq_bf = const.tile([128, H, 2, D], bf16)
        wk_bf = const.tile([128, H, 2, D], bf16)
        wv_bf = const.tile([128, H, 2, D], bf16)
        wg_bf = const.tile([128, H, 2, D], bf16)
        for wd, wb_dst in ((wq_d, wq_bf), (wk_d, wk_bf), (wv_d, wv_bf), (wg_d, wg_bf)):
            nc.gpsimd.dma_start(
                out=wb_dst,
                in_=wd.ap().rearrange("h (c p) e -> p h c e", p=128),
            )
        mask_i = const.tile([128, NCH], i32)
        nc.sync.dma_start(out=mask_i, in_=mask_d.ap().rearrange("(c p) -> p c", p=128))

        eps_t = const.tile([128, 1], f32)
        nc.vector.memset(eps_t, EPS)
        zero_t = const.tile([128, 1], f32)
        nc.vector.memset(zero_t, 0.0)
        ident = const.tile([128, 128], bf16)
        make_identity(nc, ident)

        lng_bc = const.tile([128, D], f32)
        nc.scalar.dma_start(out=lng_bc, in_=bcast_ap(lng_d.ap()))
        lnb_bc = const.tile([128, D], f32)
        nc.scalar.dma_start(out=lnb_bc, in_=bcast_ap(lnb_d.ap()))

        # ---- stage 1: first layernorm + xnT (fully per-chunk pipelined,
        # transposes on the tensor engine: no DMA-xbar mode switches)
        xn_full = big.tile([128, NCH, D * H], bf16, tag="xz")
        xn = xn_full[:, :, 0:D]
        xnT = const.tile([128, 2, N], bf16)  # [p, dc, n] = xn^T[128*dc+p, n]
        x_bf = const.tile([128, NCH, D], bf16)
        for c in range(NCH):
            st6 = small.tile([128, 6], f32, tag="st6")
            nc.vector.bn_stats(out=st6, in_=x_sb[:, c, :])
            mv = small.tile([128, 2], f32, tag="mv")
            nc.vector.bn_aggr(out=mv, in_=st6)
            rs = small.tile([128, 1], f32, tag="rs")
            nc.scalar.activation(
                out=rs, in_=mv[:, 1:2], func=AF.Sqrt, bias=eps_t[:], scale=1.0
            )
            nc.vector.reciprocal(rs, rs)
            t1 = small.tile([128, D], bf16, tag="lnt")
            nc.vector.scalar_tensor_tensor(
                out=t1, in0=x_sb[:, c, :], scalar=mv[:, 0:1], in1=lng_bc,
                op0=OP.subtract, op1=OP.mult,
            )
            nc.vector.scalar_tensor_tensor(
                out=xn[:, c, :], in0=t1, scalar=rs, in1=lnb_bc,
                op0=OP.mult, op1=OP.add,
            )
            for dc in range(2):
                tr_ps = ps_vg.tile([128, 512], bf16, tag="pvg")
                nc.tensor.transpose(
                    tr_ps[:, 0:128], xn[:, c, 128 * dc : 128 * dc + 128], ident
                )
                nc.any.tensor_copy(
                    out=xnT[:, dc, 128 * c : 128 * c + 128], in_=tr_ps[:, 0:128]
                )
            nc.any.tensor_copy(out=x_bf[:, c, :], in_=x_sb[:, c, :])

        mask_f = const.tile([128, NCH], f32)
        nc.vector.tensor_copy(out=mask_f, in_=mask_i)
        m_half = const.tile([128, NCH], f32)
        nc.vector.tensor_scalar_mul(m_half, mask_f, 0.5)
        mask_bf = const.tile([128, NCH], bf16)
        nc.any.tensor_copy(out=mask_bf, in_=mask_f)


        # ---- stage 2: heads
        t_all = big.tile([128, H, NCH, D], bf16, tag="tz")
        mv_r = big.tile([128, H, NCH, 2], f32)
        z = big.tile([128, NCH, D * H], bf16, tag="xz")  # [p(n), c, h*256+e]
        zT = big.tile([128, NCH, N], bf16)  # [p, kc, n] = z^T[128*kc+p, n]
        y_sb = big.tile([128, NCH, D], f32)
        y_out = big.tile([128, NCH, D], f32)

        def tail_prep_chunk(c):
            # lnr-normalize + transpose for one token chunk; called inside
            # head 3's av loop so this DVE/DMA work hides under av matmuls
            rs4 = small.tile([128, 4], f32, tag="rs4")
            nc.scalar.activation(
                out=rs4, in_=mv_r[:, :, c, 1], func=AF.Sqrt, bias=eps_t[:], scale=1.0
            )
            nc.vector.reciprocal(rs4, rs4)
            for h in range(H):
                nc.vector.tensor_scalar(
                    z[:, c, D * h : D * (h + 1)],
                    t_all[:, h, c, :],
                    mv_r[:, h, c, 0:1],
                    rs4[:, h : h + 1],
                    OP.subtract,
                    OP.mult,
                )
            eng = nc.sync if c % 2 == 0 else nc.scalar
            eng.dma_start_transpose(
                out=zT[:, :, 128 * c : 128 * c + 128], in_=z[:, c, :]
            )

        for h in range(H):
            # q^T, k^T = [e, n] projections (weights stay natural: no
            # weight transposes needed)
            qT_bf = hpool.tile([128, 2, N], bf16, tag="qT")
            kT_bf = hpool.tile([128, 2, N], bf16, tag="kT")
            for wsrc, tdst in ((wq_bf, qT_bf), (wk_bf, kT_bf)):
                for ec in range(2):
                    for nh in range(2):
                        p_ps = ps_vg.tile([128, 512], f32, tag="pvg")
                        for kd in range(2):
                            nc.tensor.matmul(
                                p_ps,
                                lhsT=wsrc[:, h, kd, 128 * ec : 128 * ec + 128],
                                rhs=xnT[:, kd, 512 * nh : 512 * nh + 512],
                                start=(kd == 0),
                                stop=(kd == 1),
                            )
                        nc.any.tensor_copy(
                            out=tdst[:, ec, 512 * nh : 512 * nh + 512], in_=p_ps
                        )

            # v'' = [xn @ wv * m_k | m_k]
            v2 = hpool.tile([128, NCH, D + 1], bf16, tag="v2")
            for mc in range(NCH):
                v_ps = ps_vg.tile([128, 512], f32, tag="pvg")
                for kd in range(2):
                    nc.tensor.matmul(
                        v_ps[:, 0:D],
                        lhsT=xnT[:, kd, 128 * mc : 128 * mc + 128],
                        rhs=wv_bf[:, h, kd, :],
                        start=(kd == 0),
                        stop=(kd == 1),
                    )
                nc.any.tensor_scalar(
                    v2[:, mc, 0:D], v_ps[:, 0:D], mask_f[:, mc : mc + 1], None, OP.mult
                )
            nc.any.tensor_copy(out=v2[:, :, D], in_=mask_bf)

            # gate pre-activation: tanh(0.5 * xn @ wg)
            tanh_o = hpool.tile([128, NCH, D], bf16, tag="tanh")
            for c in range(NCH):
                g_ps = ps_vg.tile([128, 512], f32, tag="pvg")
                for kd in range(2):
                    nc.tensor.matmul(
                        g_ps[:, 0:D],
                        lhsT=xnT[:, kd, 128 * c : 128 * c + 128],
                        rhs=wg_bf[:, h, kd, :],
                        start=(kd == 0),
                        stop=(kd == 1),
                    )
                nc.scalar.activation(
                    out=tanh_o[:, c, :], in_=g_ps[:, 0:D], func=AF.Tanh,
                    bias=zero_t[:], scale=0.5,
                )

            # logits transposed: s^T tiles [m-chunk, n]; exp output is the
            # av lhsT layout directly (no transpose)
            esT_tiles = []
            for mc in range(NCH):
                s_ps = ps_s.tile([128, N], f32, tag="s")
                for kc in range(2):
                    for nh in range(2):
                        nc.tensor.matmul(
                            s_ps[:, 512 * nh : 512 * nh + 512],
                            lhsT=kT_bf[:, kc, 128 * mc : 128 * mc + 128],
                            rhs=qT_bf[:, kc, 512 * nh : 512 * nh + 512],
                            start=(kc == 0),
                            stop=(kc == 1),
                        )
                esT = spool.tile([128, N], bf16, tag="esT")
                nc.scalar.activation(
                    out=esT, in_=s_ps, func=AF.Exp, bias=zero_t[:], scale=SCALE
                )
                esT_tiles.append(esT)

            for c in range(NCH):
                o_ps = ps_o.tile([128, D + 1], f32, tag="o")
                for mc in range(NCH):
                    nc.tensor.matmul(
                        o_ps,
                        lhsT=esT_tiles[mc][:, 128 * c : 128 * c + 128],
                        rhs=v2[:, mc, :],
                        start=(mc == 0),
                        stop=(mc == NCH - 1),
                    )
                hf = small.tile([128, 1], f32, tag="hf")
                nc.vector.reciprocal(hf, o_ps[:, D : D + 1])
                nc.vector.tensor_scalar_mul(hf, hf, m_half[:, c : c + 1])
                tmp = small.tile([128, D], bf16, tag="tmp")
                nc.vector.scalar_tensor_tensor(
                    out=tmp,
                    in0=tanh_o[:, c, :],
                    scalar=1.0,
                    in1=o_ps[:, 0:D],
                    op0=OP.add,
                    op1=OP.mult,
                )
                nc.vector.scalar_tensor_tensor(
                    out=t_all[:, h, c, :],
                    in0=tmp,
                    scalar=hf,
                    in1=x_bf[:, c, :],
                    op0=OP.mult,
                    op1=OP.add,
                )
                st6 = small.tile([128, 6], f32, tag="st6")
                nc.vector.bn_stats(out=st6, in_=t_all[:, h, c, :])
                nc.vector.bn_aggr(out=mv_r[:, h, c, :], in_=st6)
                if h == H - 1:
                    tail_prep_chunk(c)

            if h == 1:
                # out_w / bias prep emitted mid-kernel: DMAs overlap head
                # compute, results only needed at the tail
                gcol = const.tile([128, 2], f32)
                nc.gpsimd.dma_start(
                    out=gcol, in_=lnrg_d.ap().rearrange("(b p) -> p b", p=128)
                )
                bcol_bf = const.tile([128, 2], bf16)
                nc.gpsimd.dma_start(
                    out=bcol_bf, in_=lnrb_d.ap().rearrange("(b p) -> p b", p=128)
                )
                # out_w permuted to [p, h, b, col] (row (128b+p)*4+h), bf16 cast
                wo_raw = const.tile([128, H, 2, D], bf16)
                nc.gpsimd.dma_start(
                    out=wo_raw,
                    in_=ow_d.ap().rearrange("(b p h) o -> p h b o", b=2, p=128, h=H),
                )
                wo_bf = const.tile([128, H, 2, D], bf16)
                for hh in range(H):
                    for b2 in range(2):
                        nc.any.tensor_scalar(
                            wo_bf[:, hh, b2, :],
                            wo_raw[:, hh, b2, :],
                            gcol[:, b2 : b2 + 1],
                            None,
                            OP.mult,
                        )
                ob_row = const.tile([1, D], f32)
                ob_ap = ob_d.ap()
                nc.gpsimd.dma_start(
                    out=ob_row,
                    in_=bass.AP(
                        tensor=ob_ap.tensor, offset=ob_ap.offset,
                        ap=[[0, 1]] + list(ob_ap.ap),
                    ),
                )
                lnog_bc = const.tile([128, D], f32)
                nc.gpsimd.dma_start(out=lnog_bc, in_=bcast_ap(lnog_d.ap()))
                lnob_bc = const.tile([128, D], f32)
                nc.gpsimd.dma_start(out=lnob_bc, in_=bcast_ap(lnob_d.ap()))

            if h == 2:
                # bias row = out_b + lnr_b @ out_w, broadcast via DRAM
                bias_ps = ps_o.tile([1, D], f32, tag="o")
                i = 0
                for b2 in range(2):
                    for hh in range(H):
                        nc.tensor.matmul(
                            bias_ps,
                            lhsT=bcol_bf[:, b2 : b2 + 1],
                            rhs=wo_raw[:, hh, b2, :],
                            start=(i == 0),
                            stop=(i == 7),
                        )
                        i += 1
                bias_row = const.tile([1, D], f32)
                nc.vector.tensor_add(bias_row, bias_ps, ob_row)
                bias_dram = nc.dram_tensor("bias_scratch", [D], f32, kind="Internal")
                nc.gpsimd.dma_start(
                    out=bias_dram.ap().rearrange("(o d) -> o d", o=1), in_=bias_row
                )
                bias_bc = const.tile([128, D], f32)
                nc.gpsimd.dma_start(out=bias_bc, in_=bcast_ap(bias_dram.ap()))
                xb = const.tile([128, NCH, D], f32)
                for c in range(NCH):
                    nc.any.tensor_add(xb[:, c, :], x_sb[:, c, :], bias_bc)

        for c in range(NCH):
            y_ps = ps_s.tile([128, D], f32, tag="s")
            for kc in range(NCH):
                nc.tensor.matmul(
                    y_ps,
                    lhsT=zT[:, kc, 128 * c : 128 * c + 128],
                    rhs=wo_bf[:, kc // 2, kc % 2, :],
                    start=(kc == 0),
                    stop=(kc == NCH - 1),
                )
            nc.any.tensor_add(y_sb[:, c, :], y_ps, xb[:, c, :])
            st6 = small.tile([128, 6], f32, tag="st6")
            nc.vector.bn_stats(out=st6, in_=y_sb[:, c, :])
            mvo = small.tile([128, 2], f32, tag="mv")
            nc.vector.bn_aggr(out=mvo, in_=st6)
            rso = small.tile([128, 1], f32, tag="rs")
            nc.scalar.activation(
                out=rso, in_=mvo[:, 1:2], func=AF.Sqrt, bias=eps_t[:], scale=1.0
            )
            nc.vector.reciprocal(rso, rso)
            f1 = small.tile([128, D], f32, tag="f1")
            nc.vector.scalar_tensor_tensor(
                out=f1, in0=y_sb[:, c, :], scalar=mvo[:, 0:1], in1=lnog_bc,
                op0=OP.subtract, op1=OP.mult,
            )
            f2 = small.tile([128, D], f32, tag="f2")
            nc.vector.scalar_tensor_tensor(
                out=f2, in0=f1, scalar=rso, in1=lnob_bc,
                op0=OP.mult, op1=OP.add,
            )
            nc.any.tensor_scalar(
                y_out[:, c, :], f2, mask_f[:, c : c + 1], None, OP.mult
            )
        nc.sync.dma_start(
            out=y_d.ap().rearrange("(c p) d -> p c d", p=128), in_=y_out
        )

    nc.compile()
    return nc


def _get_program():
    global _PROGRAM
    if _PROGRAM is None:
        _PROGRAM = _build_program()
    return _PROGRAM


def _make_in_maps(inputs):
    full = {k: np.asarray(v) for k, v in inputs.items()}
    in_maps = []
    for b in range(8):
        m = {
            "x": np.ascontiguousarray(full["x"][b], dtype=np.float32),
            "mask": np.ascontiguousarray(full["mask"][b], dtype=np.int32),
        }
        for k in ("wq", "wk", "wv", "wg", "out_w", "out_b", "ln_g", "ln_b",
                  "lnr_g", "lnr_b", "lno_g", "lno_b"):
            m[k] = np.ascontiguousarray(full[k], dtype=np.float32)
        in_maps.append(m)
    return in_maps


def run_on_hw(inputs, trace=False):
    """Run on the 8 NeuronCores; returns (output [8,1024,256] f32, results obj)."""
    from concourse import bass_utils

    nc = _get_program()
    in_maps = _make_in_maps(inputs)
    res = bass_utils.run_bass_kernel_spmd(
        nc, in_maps, core_ids=list(range(8)), trace=trace
    )
    out = np.stack([res.results[b]["y"] for b in range(8)], axis=0).astype(np.float32)
    return out, res


def kernel(**inputs) -> np.ndarray:
    out, _ = run_on_hw(inputs, trace=False)
    return out


# revision 20
# speedup vs baseline: 1.2662x; 1.0083x over previous
"""Trainium2 Bass kernel for nn_AttentionLayer (B=8, N=1024, D=256, H=4).

Sharding: pure data-parallel over batch B across 8 NeuronCores (one batch
element per core, all parameters replicated). No collectives.

Per-core algorithm (bf16 matmuls, fp32 stats/output):
  x_norm = LN(x)                               (bn_stats, free-dim LN)
  xnT    = x_norm^T                            (DMA xbar transpose, bf16)
  per head h:
    A_h  = wq_h @ wk_h^T                       (256x256 -> s = xn A xn^T)
    B^T  = [d', n] = A-contract vs xnT
    s^T  = xnT-chunk.T @ B^T                   ([m, n] logits in PSUM)
    esT  = exp(s^T/16)                         (ACT; already av-lhsT layout)
    v''  = [(xn @ wv_h) * m_k | m_k]           ([m, 257], key mask folded)
    out  = esT.T @ v''                         ([n, 257]; col 256 = denom)
    gate = sigmoid via tanh                    (ACT tanh + fused affine)
    t_h  = out * gate * (0.5*m_q/denom) + x    (fused scalar_tensor_tensor)
  z     = concat_h LN_lnr(t_h)   (gamma folded into out_w; beta via bias row)
  y     = zT.T @ out_w' + (out_b + beta_r@out_w) + x
  out   = LN_lno(y) * mask
Weights stream in via SWDGE cast-DMAs (f32->bf16 during transfer); wq/wk are
transposed on the tensor engine at startup (also warms HAM).
"""

import os
import sys

for _p in ("/opt/trn_rl_repo", "/root/.axon_site/_ro/trn_rl_repo"):
    if os.path.isdir(_p) and _p not in sys.path:
        sys.path.insert(0, _p)
        break

import numpy as np

N, D, H = 1024, 256, 4
NCH = N // 128  # 8 token chunks
EPS = 1e-6
SCALE = 1.0 / 16.0

_PROGRAM = None  # built Bass program, cached across kernel() calls


def _build_program():
    from contextlib import ExitStack

    import concourse.bass as bass
    import concourse.mybir as mybir
    import concourse.tile as tile
    from concourse import bacc
    from concourse.masks import make_identity

    f32 = mybir.dt.float32
    bf16 = mybir.dt.bfloat16
    i32 = mybir.dt.int32
    AF = mybir.ActivationFunctionType
    OP = mybir.AluOpType

    nc = bacc.Bacc(
        "TRN2",
        target_bir_lowering=False,
        debug=False,
        enable_asserts=False,
        num_devices=8,
    )

    x_d = nc.dram_tensor("x", [N, D], f32, kind="ExternalInput")
    mask_d = nc.dram_tensor("mask", [N], i32, kind="ExternalInput")
    wq_d = nc.dram_tensor("wq", [H, D, D], f32, kind="ExternalInput")
    wk_d = nc.dram_tensor("wk", [H, D, D], f32, kind="ExternalInput")
    wv_d = nc.dram_tensor("wv", [H, D, D], f32, kind="ExternalInput")
    wg_d = nc.dram_tensor("wg", [H, D, D], f32, kind="ExternalInput")
    ow_d = nc.dram_tensor("out_w", [D * H, D], f32, kind="ExternalInput")
    ob_d = nc.dram_tensor("out_b", [D], f32, kind="ExternalInput")
    lng_d = nc.dram_tensor("ln_g", [D], f32, kind="ExternalInput")
    lnb_d = nc.dram_tensor("ln_b", [D], f32, kind="ExternalInput")
    lnrg_d = nc.dram_tensor("lnr_g", [D], f32, kind="ExternalInput")
    lnrb_d = nc.dram_tensor("lnr_b", [D], f32, kind="ExternalInput")
    lnog_d = nc.dram_tensor("lno_g", [D], f32, kind="ExternalInput")
    lnob_d = nc.dram_tensor("lno_b", [D], f32, kind="ExternalInput")
    y_d = nc.dram_tensor("y", [N, D], f32, kind="ExternalOutput")

    def bcast_ap(ap, parts=128):
        return bass.AP(
            tensor=ap.tensor, offset=ap.offset, ap=[[0, parts]] + list(ap.ap)
        )

    with tile.TileContext(nc) as tc, ExitStack() as ctx:
        const = ctx.enter_context(tc.tile_pool(name="const", bufs=1))
        big = ctx.enter_context(tc.tile_pool(name="big", bufs=1))
        hpool = ctx.enter_context(tc.tile_pool(name="hpool", bufs=2))
        spool = ctx.enter_context(tc.tile_pool(name="spool", bufs=12))
        small = ctx.enter_context(tc.tile_pool(name="small", bufs=3))
        ps_s = ctx.enter_context(tc.tile_pool(name="ps_s", bufs=2, space="PSUM"))
        ps_o = ctx.enter_context(tc.tile_pool(name="ps_o", bufs=2, space="PSUM"))
        ps_vg = ctx.enter_context(tc.tile_pool(name="ps_vg", bufs=2, space="PSUM"))

        # ---- stage 0a: x / mask on the sync ring (per chunk: LN starts early)
        # identity first on Q7 (gates the xn transposes), then x on the sync
        # HWDGE ring (descriptor gen is immediate there, so x beats the
        # SWDGE weight traffic to HBM), weights behind on SWDGE
        ident = const.tile([128, 128], bf16)
        make_identity(nc, ident)
        x_sb = const.tile([128, NCH, D], f32)
        for c in range(NCH):
            nc.sync.dma_start(
                out=x_sb[:, c, :], in_=x_d.ap()[128 * c : 128 * (c + 1), :]
            )
        mask_i = const.tile([128, NCH], i32)
        nc.sync.dma_start(out=mask_i, in_=mask_d.ap().rearrange("(c p) -> p c", p=128))
        wq_bf = const.tile([128, H, 2, D], bf16)
        wk_bf = const.tile([128, H, 2, D], bf16)
        wv_bf = const.tile([128, H, 2, D], bf16)
        wg_bf = const.tile([128, H, 2, D], bf16)
        for wd, wb_dst in ((wq_d, wq_bf), (wk_d, wk_bf), (wv_d, wv_bf), (wg_d, wg_bf)):
            nc.gpsimd.dma_start(
                out=wb_dst,
                in_=wd.ap().rearrange("h (c p) e -> p h c e", p=128),
            )

        eps_t = const.tile([128, 1], f32)
        nc.vector.memset(eps_t, EPS)
        zero_t = const.tile([128, 1], f32)
        nc.vector.memset(zero_t, 0.0)

        lng_bc = const.tile([128, D], f32)
        nc.scalar.dma_start(out=lng_bc, in_=bcast_ap(lng_d.ap()))
        lnb_bc = const.tile([128, D], f32)
        nc.scalar.dma_start(out=lnb_bc, in_=bcast_ap(lnb_d.ap()))

        # ---- stage 1: first layernorm + xnT (fully per-chunk pipelined,
        # transposes on the tensor engine: no DMA-xbar mode switches)
        xn_full = big.tile([128, NCH, D * H], bf16, tag="xz")
        xn = xn_full[:, :, 0:D]
        xnT = const.tile([128, 2, N], bf16)  # [p, dc, n] = xn^T[128*dc+p, n]
        x_bf = const.tile([128, NCH, D], bf16)
        for c in range(NCH):
            st6 = small.tile([128, 6], f32, tag="st6")
            nc.vector.bn_stats(out=st6, in_=x_sb[:, c, :])
            mv = small.tile([128, 2], f32, tag="mv")
            nc.vector.bn_aggr(out=mv, in_=st6)
            rs = small.tile([128, 1], f32, tag="rs")
            nc.scalar.activation(
                out=rs, in_=mv[:, 1:2], func=AF.Sqrt, bias=eps_t[:], scale=1.0
            )
            nc.vector.reciprocal(rs, rs)
            t1 = small.tile([128, D], bf16, tag="lnt")
            nc.vector.scalar_tensor_tensor(
                out=t1, in0=x_sb[:, c, :], scalar=mv[:, 0:1], in1=lng_bc,
                op0=OP.subtract, op1=OP.mult,
            )
            nc.vector.scalar_tensor_tensor(
                out=xn[:, c, :], in0=t1, scalar=rs, in1=lnb_bc,
                op0=OP.mult, op1=OP.add,
            )
            for dc in range(2):
                tr_ps = ps_vg.tile([128, 512], bf16, tag="pvg")
                nc.tensor.transpose(
                    tr_ps[:, 0:128], xn[:, c, 128 * dc : 128 * dc + 128], ident
                )
                nc.any.tensor_copy(
                    out=xnT[:, dc, 128 * c : 128 * c + 128], in_=tr_ps[:, 0:128]
                )
            nc.any.tensor_copy(out=x_bf[:, c, :], in_=x_sb[:, c, :])

        mask_f = const.tile([128, NCH], f32)
        nc.vector.tensor_copy(out=mask_f, in_=mask_i)
        m_half = const.tile([128, NCH], f32)
        nc.vector.tensor_scalar_mul(m_half, mask_f, 0.5)
        mask_bf = const.tile([128, NCH], bf16)
        nc.any.tensor_copy(out=mask_bf, in_=mask_f)


        # ---- stage 2: heads
        t_all = big.tile([128, H, NCH, D], bf16, tag="tz")
        mv_r = big.tile([128, H, NCH, 2], f32)
        z = big.tile([128, NCH, D * H], bf16, tag="xz")  # [p(n), c, h*256+e]
        zT = big.tile([128, NCH, N], bf16)  # [p, kc, n] = z^T[128*kc+p, n]
        y_sb = big.tile([128, NCH, D], f32)
        y_out = big.tile([128, NCH, D], f32)

        def tail_prep_chunk(c):
            # lnr-normalize + transpose for one token chunk; called inside
            # head 3's av loop so this DVE/DMA work hides under av matmuls
            rs4 = small.tile([128, 4], f32, tag="rs4")
            nc.scalar.activation(
                out=rs4, in_=mv_r[:, :, c, 1], func=AF.Sqrt, bias=eps_t[:], scale=1.0
            )
            nc.vector.reciprocal(rs4, rs4)
            for h in range(H):
                nc.vector.tensor_scalar(
                    z[:, c, D * h : D * (h + 1)],
                    t_all[:, h, c, :],
                    mv_r[:, h, c, 0:1],
                    rs4[:, h : h + 1],
                    OP.subtract,
                    OP.mult,
                )
            eng = nc.sync if c % 2 == 0 else nc.scalar
            eng.dma_start_transpose(
                out=zT[:, :, 128 * c : 128 * c + 128], in_=z[:, c, :]
            )

        for h in range(H):
            # q^T, k^T = [e, n] projections (weights stay natural: no
            # weight transposes needed)
            qT_bf = hpool.tile([128, 2, N], bf16, tag="qT")
            kT_bf = hpool.tile([128, 2, N], bf16, tag="kT")
            for wsrc, tdst in ((wq_bf, qT_bf), (wk_bf, kT_bf)):
                for ec in range(2):
                    for nh in range(2):
                        p_ps = ps_vg.tile([128, 512], f32, tag="pvg")
                        for kd in range(2):
                            nc.tensor.matmul(
                                p_ps,
                                lhsT=wsrc[:, h, kd, 128 * ec : 128 * ec + 128],
                                rhs=xnT[:, kd, 512 * nh : 512 * nh + 512],
                                start=(kd == 0),
                                stop=(kd == 1),
                            )
                        nc.any.tensor_copy(
                            out=tdst[:, ec, 512 * nh : 512 * nh + 512], in_=p_ps
                        )

            # v'' = [xn @ wv * m_k | m_k]
            v2 = hpool.tile([128, NCH, D + 1], bf16, tag="v2")
            for mc in range(NCH):
                v_ps = ps_vg.tile([128, 512], f32, tag="pvg")
                for kd in range(2):
                    nc.tensor.matmul(
                        v_ps[:, 0:D],
                        lhsT=xnT[:, kd, 128 * mc : 128 * mc + 128],
                        rhs=wv_bf[:, h, kd, :],
                        start=(kd == 0),
                        stop=(kd == 1),
                    )
                nc.any.tensor_scalar(
                    v2[:, mc, 0:D], v_ps[:, 0:D], mask_f[:, mc : mc + 1], None, OP.mult
                )
            nc.any.tensor_copy(out=v2[:, :, D], in_=mask_bf)

            # gate pre-activation: tanh(0.5 * xn @ wg)
            tanh_o = hpool.tile([128, NCH, D], bf16, tag="tanh")
            for c in range(NCH):
                g_ps = ps_vg.tile([128, 512], f32, tag="pvg")
                for kd in range(2):
                    nc.tensor.matmul(
                        g_ps[:, 0:D],
                        lhsT=xnT[:, kd, 128 * c : 128 * c + 128],
                        rhs=wg_bf[:, h, kd, :],
                        start=(kd == 0),
                        stop=(kd == 1),
                    )
                nc.scalar.activation(
                    out=tanh_o[:, c, :], in_=g_ps[:, 0:D], func=AF.Tanh,
                    bias=zero_t[:], scale=0.5,
                )

            # logits transposed: s^T tiles [m-chunk, n]; exp output is the
            # av lhsT layout directly (no transpose)
            esT_tiles = []
            for mc in range(NCH):
                s_ps = ps_s.tile([128, N], f32, tag="s")
                for kc in range(2):
                    for nh in range(2):
                        nc.tensor.matmul(
                            s_ps[:, 512 * nh : 512 * nh + 512],
                            lhsT=kT_bf[:, kc, 128 * mc : 128 * mc + 128],
                            rhs=qT_bf[:, kc, 512 * nh : 512 * nh + 512],
                            start=(kc == 0),
                            stop=(kc == 1),
                        )
                esT = spool.tile([128, N], bf16, tag="esT")
                nc.scalar.activation(
                    out=esT, in_=s_ps, func=AF.Exp, bias=zero_t[:], scale=SCALE
                )
                esT_tiles.append(esT)

            for c in range(NCH):
                o_ps = ps_o.tile([128, D + 1], f32, tag="o")
                for mc in range(NCH):
                    nc.tensor.matmul(
                        o_ps,
                        lhsT=esT_tiles[mc][:, 128 * c : 128 * c + 128],
                        rhs=v2[:, mc, :],
                        start=(mc == 0),
                        stop=(mc == NCH - 1),
                    )
                hf = small.tile([128, 1], f32, tag="hf")
                nc.vector.reciprocal(hf, o_ps[:, D : D + 1])
                nc.vector.tensor_scalar_mul(hf, hf, m_half[:, c : c + 1])
                tmp = small.tile([128, D], bf16, tag="tmp")
                nc.vector.scalar_tensor_tensor(
                    out=tmp,
                    in0=tanh_o[:, c, :],
                    scalar=1.0,
                    in1=o_ps[:, 0:D],
                    op0=OP.add,
                    op1=OP.mult,
                )
                nc.vector.scalar_tensor_tensor(
                    out=t_all[:, h, c, :],
                    in0=tmp,
                    scalar=hf,
                    in1=x_bf[:, c, :],
                    op0=OP.mult,
                    op1=OP.add,
                )
                st6 = small.tile([128, 6], f32, tag="st6")
                nc.vector.bn_stats(out=st6, in_=t_all[:, h, c, :])
                nc.vector.bn_aggr(out=mv_r[:, h, c, :], in_=st6)
                if h == H - 1:
                    tail_prep_chunk(c)

            if h == 1:
                # out_w / bias prep emitted mid-kernel: DMAs overlap head
                # compute, results only needed at the tail
                gcol = const.tile([128, 2], f32)
                nc.gpsimd.dma_start(
                    out=gcol, in_=lnrg_d.ap().rearrange("(b p) -> p b", p=128)
                )
                bcol_bf = const.tile([128, 2], bf16)
                nc.gpsimd.dma_start(
                    out=bcol_bf, in_=lnrb_d.ap().rearrange("(b p) -> p b", p=128)
                )
                # out_w permuted to [p, h, b, col] (row (128b+p)*4+h), bf16 cast
                wo_raw = const.tile([128, H, 2, D], bf16)
                nc.gpsimd.dma_start(
                    out=wo_raw,
                    in_=ow_d.ap().rearrange("(b p h) o -> p h b o", b=2, p=128, h=H),
                )
                wo_bf = const.tile([128, H, 2, D], bf16)
                for hh in range(H):
                    for b2 in range(2):
                        nc.any.tensor_scalar(
                            wo_bf[:, hh, b2, :],
                            wo_raw[:, hh, b2, :],
                            gcol[:, b2 : b2 + 1],
                            None,
                            OP.mult,
                        )
                ob_row = const.tile([1, D], f32)
                ob_ap = ob_d.ap()
                nc.gpsimd.dma_start(
                    out=ob_row,
                    in_=bass.AP(
                        tensor=ob_ap.tensor, offset=ob_ap.offset,
                        ap=[[0, 1]] + list(ob_ap.ap),
                    ),
                )
                lnog_bc = const.tile([128, D], f32)
                nc.gpsimd.dma_start(out=lnog_bc, in_=bcast_ap(lnog_d.ap()))
                lnob_bc = const.tile([128, D], f32)
                nc.gpsimd.dma_start(out=lnob_bc, in_=bcast_ap(lnob_d.ap()))

            if h == 2:
                # bias row = out_b + lnr_b @ out_w, broadcast via DRAM
                bias_ps = ps_o.tile([1, D], f32, tag="o")
                i = 0
                for b2 in range(2):
                    for hh in range(H):
                        nc.tensor.matmul(
                            bias_ps,
                            lhsT=bcol_bf[:, b2 : b2 + 1],
                            rhs=wo_raw[:, hh, b2, :],
                            start=(i == 0),
                            stop=(i == 7),
                        )
                        i += 1
                bias_row = const.tile([1, D], f32)
                nc.vector.tensor_add(bias_row, bias_ps, ob_row)
                bias_dram = nc.dram_tensor("bias_scratch", [D], f32, kind="Internal")
                nc.gpsimd.dma_start(
                    out=bias_dram.ap().rearrange("(o d) -> o d", o=1), in_=bias_row
                )
                bias_bc = const.tile([128, D], f32)
                nc.gpsimd.dma_start(out=bias_bc, in_=bcast_ap(bias_dram.ap()))
                xb = const.tile([128, NCH, D], f32)
                for c in range(NCH):
                    nc.any.tensor_add(xb[:, c, :], x_sb[:, c, :], bias_bc)

        for c in range(NCH):
            y_ps = ps_s.tile([128, D], f32, tag="s")
            for kc in range(NCH):
                nc.tensor.matmul(
                    y_ps,
                    lhsT=zT[:, kc, 128 * c : 128 * c + 128],
                    rhs=wo_bf[:, kc // 2, kc % 2, :],
                    start=(kc == 0),
                    stop=(kc == NCH - 1),
                )
            nc.any.tensor_add(y_sb[:, c, :], y_ps, xb[:, c, :])
            st6 = small.tile([128, 6], f32, tag="st6")
            nc.vector.bn_stats(out=st6, in_=y_sb[:, c, :])
            mvo = small.tile([128, 2], f32, tag="mv")
            nc.vector.bn_aggr(out=mvo, in_=st6)
            rso = small.tile([128, 1], f32, tag="rs")
            nc.scalar.activation(
                out=rso, in_=mvo[:, 1:2], func=AF.Sqrt, bias=eps_t[:], scale=1.0
            )
            nc.vector.reciprocal(rso, rso)
            f1 = small.tile([128, D], f32, tag="f1")
            nc.vector.scalar_tensor_tensor(
                out=f1, in0=y_sb[:, c, :], scalar=mvo[:, 0:1], in1=lnog_bc,
                op0=OP.subtract, op1=OP.mult,
            )
            f2 = small.tile([128, D], f32, tag="f2")
            nc.vector.scalar_tensor_tensor(
                out=f2, in0=f1, scalar=rso, in1=lnob_bc,
                op0=OP.mult, op1=OP.add,
            )
            nc.any.tensor_scalar(
                y_out[:, c, :], f2, mask_f[:, c : c + 1], None, OP.mult
            )
        nc.sync.dma_start(
            out=y_d.ap().rearrange("(c p) d -> p c d", p=128), in_=y_out
        )

    nc.compile()
    return nc


def _get_program():
    global _PROGRAM
    if _PROGRAM is None:
        _PROGRAM = _build_program()
    return _PROGRAM


def _make_in_maps(inputs):
    full = {k: np.asarray(v) for k, v in inputs.items()}
    in_maps = []
    for b in range(8):
        m = {
            "x": np.ascontiguousarray(full["x"][b], dtype=np.float32),
            "mask": np.ascontiguousarray(full["mask"][b], dtype=np.int32),
        }
        for k in ("wq", "wk", "wv", "wg", "out_w", "out_b", "ln_g", "ln_b",
                  "lnr_g", "lnr_b", "lno_g", "lno_b"):
            m[k] = np.ascontiguousarray(full[k], dtype=np.float32)
        in_maps.append(m)
    return in_maps


def run_on_hw(inputs, trace=False):
    """Run on the 8 NeuronCores; returns (output [8,1024,256] f32, results obj)."""
    from concourse import bass_utils

    nc = _get_program()
    in_maps = _make_in_maps(inputs)
    res = bass_utils.run_bass_kernel_spmd(
        nc, in_maps, core_ids=list(range(8)), trace=trace
    )
    out = np.stack([res.results[b]["y"] for b in range(8)], axis=0).astype(np.float32)
    return out, res


def kernel(**inputs) -> np.ndarray:
    out, _ = run_on_hw(inputs, trace=False)
    return out


# revision 21
# speedup vs baseline: 1.2941x; 1.0221x over previous
"""Trainium2 Bass kernel for nn_AttentionLayer (B=8, N=1024, D=256, H=4).

Sharding: pure data-parallel over batch B across 8 NeuronCores (one batch
element per core, all parameters replicated). No collectives.

Per-core algorithm (bf16 matmuls, fp32 stats/output):
  x_norm = LN(x)                               (bn_stats, free-dim LN)
  xnT    = x_norm^T                            (DMA xbar transpose, bf16)
  per head h:
    A_h  = wq_h @ wk_h^T                       (256x256 -> s = xn A xn^T)
    B^T  = [d', n] = A-contract vs xnT
    s^T  = xnT-chunk.T @ B^T                   ([m, n] logits in PSUM)
    esT  = exp(s^T/16)                         (ACT; already av-lhsT layout)
    v''  = [(xn @ wv_h) * m_k | m_k]           ([m, 257], key mask folded)
    out  = esT.T @ v''                         ([n, 257]; col 256 = denom)
    gate = sigmoid via tanh                    (ACT tanh + fused affine)
    t_h  = out * gate * (0.5*m_q/denom) + x    (fused scalar_tensor_tensor)
  z     = concat_h LN_lnr(t_h)   (gamma folded into out_w; beta via bias row)
  y     = zT.T @ out_w' + (out_b + beta_r@out_w) + x
  out   = LN_lno(y) * mask
Weights stream in via SWDGE cast-DMAs (f32->bf16 during transfer); wq/wk are
transposed on the tensor engine at startup (also warms HAM).
"""

import os
import sys

for _p in ("/opt/trn_rl_repo", "/root/.axon_site/_ro/trn_rl_repo"):
    if os.path.isdir(_p) and _p not in sys.path:
        sys.path.insert(0, _p)
        break

import numpy as np

N, D, H = 1024, 256, 4
NCH = N // 128  # 8 token chunks
EPS = 1e-6
SCALE = 1.0 / 16.0

_PROGRAM = None  # built Bass program, cached across kernel() calls


def _build_program():
    from contextlib import ExitStack

    import concourse.bass as bass
    import concourse.mybir as mybir
    import concourse.tile as tile
    from concourse import bacc
    from concourse.masks import make_identity

    f32 = mybir.dt.float32
    bf16 = mybir.dt.bfloat16
    i32 = mybir.dt.int32
    AF = mybir.ActivationFunctionType
    OP = mybir.AluOpType

    nc = bacc.Bacc(
        "TRN2",
        target_bir_lowering=False,
        debug=False,
        enable_asserts=False,
        num_devices=8,
    )

    x_d = nc.dram_tensor("x", [N, D], f32, kind="ExternalInput")
    mask_d = nc.dram_tensor("mask", [N], i32, kind="ExternalInput")
    wq_d = nc.dram_tensor("wq", [H, D, D], f32, kind="ExternalInput")
    wk_d = nc.dram_tensor("wk", [H, D, D], f32, kind="ExternalInput")
    wv_d = nc.dram_tensor("wv", [H, D, D], f32, kind="ExternalInput")
    wg_d = nc.dram_tensor("wg", [H, D, D], f32, kind="ExternalInput")
    ow_d = nc.dram_tensor("out_w", [D * H, D], f32, kind="ExternalInput")
    ob_d = nc.dram_tensor("out_b", [D], f32, kind="ExternalInput")
    lng_d = nc.dram_tensor("ln_g", [D], f32, kind="ExternalInput")
    lnb_d = nc.dram_tensor("ln_b", [D], f32, kind="ExternalInput")
    lnrg_d = nc.dram_tensor("lnr_g", [D], f32, kind="ExternalInput")
    lnrb_d = nc.dram_tensor("lnr_b", [D], f32, kind="ExternalInput")
    lnog_d = nc.dram_tensor("lno_g", [D], f32, kind="ExternalInput")
    lnob_d = nc.dram_tensor("lno_b", [D], f32, kind="ExternalInput")
    y_d = nc.dram_tensor("y", [N, D], f32, kind="ExternalOutput")

    def bcast_ap(ap, parts=128):
        return bass.AP(
            tensor=ap.tensor, offset=ap.offset, ap=[[0, parts]] + list(ap.ap)
        )

    with tile.TileContext(nc) as tc, ExitStack() as ctx:
        const = ctx.enter_context(tc.tile_pool(name="const", bufs=1))
        big = ctx.enter_context(tc.tile_pool(name="big", bufs=1))
        hpool = ctx.enter_context(tc.tile_pool(name="hpool", bufs=2))
        spool = ctx.enter_context(tc.tile_pool(name="spool", bufs=12))
        small = ctx.enter_context(tc.tile_pool(name="small", bufs=3))
        ps_s = ctx.enter_context(tc.tile_pool(name="ps_s", bufs=2, space="PSUM"))
        ps_o = ctx.enter_context(tc.tile_pool(name="ps_o", bufs=2, space="PSUM"))
        ps_vg = ctx.enter_context(tc.tile_pool(name="ps_vg", bufs=2, space="PSUM"))

        # ---- stage 0a: x / mask on the sync ring (per chunk: LN starts early)
        # identity first on Q7 (gates the xn transposes), then x on the sync
        # HWDGE ring (descriptor gen is immediate there, so x beats the
        # SWDGE weight traffic to HBM), weights behind on SWDGE
        ident = const.tile([128, 128], bf16)
        make_identity(nc, ident)
        x_sb = const.tile([128, NCH, D], f32)
        nc.sync.dma_start(out=x_sb, in_=x_d.ap().rearrange("(c p) d -> p c d", p=128))
        mask_i = const.tile([128, NCH], i32)
        nc.sync.dma_start(out=mask_i, in_=mask_d.ap().rearrange("(c p) -> p c", p=128))
        wq_bf = const.tile([128, H, 2, D], bf16)
        wk_bf = const.tile([128, H, 2, D], bf16)
        wv_bf = const.tile([128, H, 2, D], bf16)
        wg_bf = const.tile([128, H, 2, D], bf16)
        for wd, wb_dst in ((wq_d, wq_bf), (wk_d, wk_bf), (wv_d, wv_bf), (wg_d, wg_bf)):
            nc.gpsimd.dma_start(
                out=wb_dst,
                in_=wd.ap().rearrange("h (c p) e -> p h c e", p=128),
            )

        eps_t = const.tile([128, 1], f32)
        nc.vector.memset(eps_t, EPS)
        zero_t = const.tile([128, 1], f32)
        nc.vector.memset(zero_t, 0.0)

        lng_bc = const.tile([128, D], f32)
        nc.scalar.dma_start(out=lng_bc, in_=bcast_ap(lng_d.ap()))
        lnb_bc = const.tile([128, D], f32)
        nc.scalar.dma_start(out=lnb_bc, in_=bcast_ap(lnb_d.ap()))

        # ---- stage 1: first layernorm + xnT (fully per-chunk pipelined,
        # transposes on the tensor engine: no DMA-xbar mode switches)
        xn_full = big.tile([128, NCH, D * H], bf16, tag="xz")
        xn = xn_full[:, :, 0:D]
        xnT = const.tile([128, 2, N], bf16)  # [p, dc, n] = xn^T[128*dc+p, n]
        x_bf = const.tile([128, NCH, D], bf16)
        for c in range(NCH):
            st6 = small.tile([128, 6], f32, tag="st6")
            nc.vector.bn_stats(out=st6, in_=x_sb[:, c, :])
            mv = small.tile([128, 2], f32, tag="mv")
            nc.vector.bn_aggr(out=mv, in_=st6)
            rs = small.tile([128, 1], f32, tag="rs")
            nc.scalar.activation(
                out=rs, in_=mv[:, 1:2], func=AF.Sqrt, bias=eps_t[:], scale=1.0
            )
            nc.vector.reciprocal(rs, rs)
            t1 = small.tile([128, D], bf16, tag="lnt")
            nc.vector.scalar_tensor_tensor(
                out=t1, in0=x_sb[:, c, :], scalar=mv[:, 0:1], in1=lng_bc,
                op0=OP.subtract, op1=OP.mult,
            )
            nc.vector.scalar_tensor_tensor(
                out=xn[:, c, :], in0=t1, scalar=rs, in1=lnb_bc,
                op0=OP.mult, op1=OP.add,
            )
            for dc in range(2):
                tr_ps = ps_vg.tile([128, 512], bf16, tag="pvg")
                nc.tensor.transpose(
                    tr_ps[:, 0:128], xn[:, c, 128 * dc : 128 * dc + 128], ident
                )
                nc.any.tensor_copy(
                    out=xnT[:, dc, 128 * c : 128 * c + 128], in_=tr_ps[:, 0:128]
                )
            nc.any.tensor_copy(out=x_bf[:, c, :], in_=x_sb[:, c, :])

        mask_f = const.tile([128, NCH], f32)
        nc.vector.tensor_copy(out=mask_f, in_=mask_i)
        m_half = const.tile([128, NCH], f32)
        nc.vector.tensor_scalar_mul(m_half, mask_f, 0.5)
        mask_bf = const.tile([128, NCH], bf16)
        nc.any.tensor_copy(out=mask_bf, in_=mask_f)


        # ---- stage 2: heads
        t_all = big.tile([128, H, NCH, D], bf16, tag="tz")
        mv_r = big.tile([128, H, NCH, 2], f32)
        z = big.tile([128, NCH, D * H], bf16, tag="xz")  # [p(n), c, h*256+e]
        zT = big.tile([128, NCH, N], bf16)  # [p, kc, n] = z^T[128*kc+p, n]
        y_sb = big.tile([128, NCH, D], f32)
        y_out = big.tile([128, NCH, D], f32)

        def tail_prep_chunk(c):
            # lnr-normalize + transpose for one token chunk; called inside
            # head 3's av loop so this DVE/DMA work hides under av matmuls
            rs4 = small.tile([128, 4], f32, tag="rs4")
            nc.scalar.activation(
                out=rs4, in_=mv_r[:, :, c, 1], func=AF.Sqrt, bias=eps_t[:], scale=1.0
            )
            nc.vector.reciprocal(rs4, rs4)
            for h in range(H):
                nc.vector.tensor_scalar(
                    z[:, c, D * h : D * (h + 1)],
                    t_all[:, h, c, :],
                    mv_r[:, h, c, 0:1],
                    rs4[:, h : h + 1],
                    OP.subtract,
                    OP.mult,
                )
            eng = nc.sync if c % 2 == 0 else nc.scalar
            eng.dma_start_transpose(
                out=zT[:, :, 128 * c : 128 * c + 128], in_=z[:, c, :]
            )

        for h in range(H):
            # q^T, k^T = [e, n] projections (weights stay natural: no
            # weight transposes needed)
            qT_bf = hpool.tile([128, 2, N], bf16, tag="qT")
            kT_bf = hpool.tile([128, 2, N], bf16, tag="kT")
            for wsrc, tdst in ((wq_bf, qT_bf), (wk_bf, kT_bf)):
                for ec in range(2):
                    for nh in range(2):
                        p_ps = ps_vg.tile([128, 512], f32, tag="pvg")
                        for kd in range(2):
                            nc.tensor.matmul(
                                p_ps,
                                lhsT=wsrc[:, h, kd, 128 * ec : 128 * ec + 128],
                                rhs=xnT[:, kd, 512 * nh : 512 * nh + 512],
                                start=(kd == 0),
                                stop=(kd == 1),
                            )
                        nc.any.tensor_copy(
                            out=tdst[:, ec, 512 * nh : 512 * nh + 512], in_=p_ps
                        )

            # v'' = [xn @ wv * m_k | m_k]
            v2 = hpool.tile([128, NCH, D + 1], bf16, tag="v2")
            for mc in range(NCH):
                v_ps = ps_vg.tile([128, 512], f32, tag="pvg")
                for kd in range(2):
                    nc.tensor.matmul(
                        v_ps[:, 0:D],
                        lhsT=xnT[:, kd, 128 * mc : 128 * mc + 128],
                        rhs=wv_bf[:, h, kd, :],
                        start=(kd == 0),
                        stop=(kd == 1),
                    )
                nc.any.tensor_scalar(
                    v2[:, mc, 0:D], v_ps[:, 0:D], mask_f[:, mc : mc + 1], None, OP.mult
                )
            nc.any.tensor_copy(out=v2[:, :, D], in_=mask_bf)

            # gate pre-activation: tanh(0.5 * xn @ wg)
            tanh_o = hpool.tile([128, NCH, D], bf16, tag="tanh")
            for c in range(NCH):
                g_ps = ps_vg.tile([128, 512], f32, tag="pvg")
                for kd in range(2):
                    nc.tensor.matmul(
                        g_ps[:, 0:D],
                        lhsT=xnT[:, kd, 128 * c : 128 * c + 128],
                        rhs=wg_bf[:, h, kd, :],
                        start=(kd == 0),
                        stop=(kd == 1),
                    )
                nc.scalar.activation(
                    out=tanh_o[:, c, :], in_=g_ps[:, 0:D], func=AF.Tanh,
                    bias=zero_t[:], scale=0.5,
                )

            # logits transposed: s^T tiles [m-chunk, n]; exp output is the
            # av lhsT layout directly (no transpose)
            esT_tiles = []
            for mc in range(NCH):
                s_ps = ps_s.tile([128, N], f32, tag="s")
                for kc in range(2):
                    for nh in range(2):
                        nc.tensor.matmul(
                            s_ps[:, 512 * nh : 512 * nh + 512],
                            lhsT=kT_bf[:, kc, 128 * mc : 128 * mc + 128],
                            rhs=qT_bf[:, kc, 512 * nh : 512 * nh + 512],
                            start=(kc == 0),
                            stop=(kc == 1),
                        )
                esT = spool.tile([128, N], bf16, tag="esT")
                nc.scalar.activation(
                    out=esT, in_=s_ps, func=AF.Exp, bias=zero_t[:], scale=SCALE
                )
                esT_tiles.append(esT)

            for c in range(NCH):
                o_ps = ps_o.tile([128, D + 1], f32, tag="o")
                for mc in range(NCH):
                    nc.tensor.matmul(
                        o_ps,
                        lhsT=esT_tiles[mc][:, 128 * c : 128 * c + 128],
                        rhs=v2[:, mc, :],
                        start=(mc == 0),
                        stop=(mc == NCH - 1),
                    )
                hf = small.tile([128, 1], f32, tag="hf")
                nc.vector.reciprocal(hf, o_ps[:, D : D + 1])
                nc.vector.tensor_scalar_mul(hf, hf, m_half[:, c : c + 1])
                tmp = small.tile([128, D], bf16, tag="tmp")
                nc.vector.scalar_tensor_tensor(
                    out=tmp,
                    in0=tanh_o[:, c, :],
                    scalar=1.0,
                    in1=o_ps[:, 0:D],
                    op0=OP.add,
                    op1=OP.mult,
                )
                nc.vector.scalar_tensor_tensor(
                    out=t_all[:, h, c, :],
                    in0=tmp,
                    scalar=hf,
                    in1=x_bf[:, c, :],
                    op0=OP.mult,
                    op1=OP.add,
                )
                st6 = small.tile([128, 6], f32, tag="st6")
                nc.vector.bn_stats(out=st6, in_=t_all[:, h, c, :])
                nc.vector.bn_aggr(out=mv_r[:, h, c, :], in_=st6)
                if h == H - 1:
                    tail_prep_chunk(c)

            if h == 1:
                # out_w / bias prep emitted mid-kernel: DMAs overlap head
                # compute, results only needed at the tail
                gcol = const.tile([128, 2], f32)
                nc.gpsimd.dma_start(
                    out=gcol, in_=lnrg_d.ap().rearrange("(b p) -> p b", p=128)
                )
                bcol_bf = const.tile([128, 2], bf16)
                nc.gpsimd.dma_start(
                    out=bcol_bf, in_=lnrb_d.ap().rearrange("(b p) -> p b", p=128)
                )
                # out_w permuted to [p, h, b, col] (row (128b+p)*4+h), bf16 cast
                wo_raw = const.tile([128, H, 2, D], bf16)
                nc.gpsimd.dma_start(
                    out=wo_raw,
                    in_=ow_d.ap().rearrange("(b p h) o -> p h b o", b=2, p=128, h=H),
                )
                wo_bf = const.tile([128, H, 2, D], bf16)
                for hh in range(H):
                    for b2 in range(2):
                        nc.any.tensor_scalar(
                            wo_bf[:, hh, b2, :],
                            wo_raw[:, hh, b2, :],
                            gcol[:, b2 : b2 + 1],
                            None,
                            OP.mult,
                        )
                ob_row = const.tile([1, D], f32)
                ob_ap = ob_d.ap()
                nc.gpsimd.dma_start(
                    out=ob_row,
                    in_=bass.AP(
                        tensor=ob_ap.tensor, offset=ob_ap.offset,
                        ap=[[0, 1]] + list(ob_ap.ap),
                    ),
                )
                lnog_bc = const.tile([128, D], f32)
                nc.gpsimd.dma_start(out=lnog_bc, in_=bcast_ap(lnog_d.ap()))
                lnob_bc = const.tile([128, D], f32)
                nc.gpsimd.dma_start(out=lnob_bc, in_=bcast_ap(lnob_d.ap()))

            if h == 2:
                # bias row = out_b + lnr_b @ out_w, broadcast via DRAM
                bias_ps = ps_o.tile([1, D], f32, tag="o")
                i = 0
                for b2 in range(2):
                    for hh in range(H):
                        nc.tensor.matmul(
                            bias_ps,
                            lhsT=bcol_bf[:, b2 : b2 + 1],
                            rhs=wo_raw[:, hh, b2, :],
                            start=(i == 0),
                            stop=(i == 7),
                        )
                        i += 1
                bias_row = const.tile([1, D], f32)
                nc.vector.tensor_add(bias_row, bias_ps, ob_row)
                bias_dram = nc.dram_tensor("bias_scratch", [D], f32, kind="Internal")
                nc.gpsimd.dma_start(
                    out=bias_dram.ap().rearrange("(o d) -> o d", o=1), in_=bias_row
                )
                bias_bc = const.tile([128, D], f32)
                nc.gpsimd.dma_start(out=bias_bc, in_=bcast_ap(bias_dram.ap()))
                xb = const.tile([128, NCH, D], f32)
                for c in range(NCH):
                    nc.any.tensor_add(xb[:, c, :], x_sb[:, c, :], bias_bc)

        for c in range(NCH):
            y_ps = ps_s.tile([128, D], f32, tag="s")
            for kc in range(NCH):
                nc.tensor.matmul(
                    y_ps,
                    lhsT=zT[:, kc, 128 * c : 128 * c + 128],
                    rhs=wo_bf[:, kc // 2, kc % 2, :],
                    start=(kc == 0),
                    stop=(kc == NCH - 1),
                )
            nc.any.tensor_add(y_sb[:, c, :], y_ps, xb[:, c, :])
            st6 = small.tile([128, 6], f32, tag="st6")
            nc.vector.bn_stats(out=st6, in_=y_sb[:, c, :])
            mvo = small.tile([128, 2], f32, tag="mv")
            nc.vector.bn_aggr(out=mvo, in_=st6)
            rso = small.tile([128, 1], f32, tag="rs")
            nc.scalar.activation(
                out=rso, in_=mvo[:, 1:2], func=AF.Sqrt, bias=eps_t[:], scale=1.0
            )
            nc.vector.reciprocal(rso, rso)
            f1 = small.tile([128, D], f32, tag="f1")
            nc.vector.scalar_tensor_tensor(
                out=f1, in0=y_sb[:, c, :], scalar=mvo[:, 0:1], in1=lnog_bc,
                op0=OP.subtract, op1=OP.mult,
            )
            f2 = small.tile([128, D], f32, tag="f2")
            nc.vector.scalar_tensor_tensor(
                out=f2, in0=f1, scalar=rso, in1=lnob_bc,
                op0=OP.mult, op1=OP.add,
            )
            nc.any.tensor_scalar(
                y_out[:, c, :], f2, mask_f[:, c : c + 1], None, OP.mult
            )
            nc.sync.dma_start(
                out=y_d.ap()[128 * c : 128 * (c + 1), :], in_=y_out[:, c, :]
            )

    nc.compile()
    return nc


def _get_program():
    global _PROGRAM
    if _PROGRAM is None:
        _PROGRAM = _build_program()
    return _PROGRAM


def _make_in_maps(inputs):
    full = {k: np.asarray(v) for k, v in inputs.items()}
    in_maps = []
    for b in range(8):
        m = {
            "x": np.ascontiguousarray(full["x"][b], dtype=np.float32),
            "mask": np.ascontiguousarray(full["mask"][b], dtype=np.int32),
        }
        for k in ("wq", "wk", "wv", "wg", "out_w", "out_b", "ln_g", "ln_b",
                  "lnr_g", "lnr_b", "lno_g", "lno_b"):
            m[k] = np.ascontiguousarray(full[k], dtype=np.float32)
        in_maps.append(m)
    return in_maps


def run_on_hw(inputs, trace=False):
    """Run on the 8 NeuronCores; returns (output [8,1024,256] f32, results obj)."""
    from concourse import bass_utils

    nc = _get_program()
    in_maps = _make_in_maps(inputs)
    res = bass_utils.run_bass_kernel_spmd(
        nc, in_maps, core_ids=list(range(8)), trace=trace
    )
    out = np.stack([res.results[b]["y"] for b in range(8)], axis=0).astype(np.float32)
    return out, res


def kernel(**inputs) -> np.ndarray:
    out, _ = run_on_hw(inputs, trace=False)
    return out
